# revision 1
# baseline (speedup 1.0000x reference)
"""Trainium2 Bass kernel for nn_CausalSelfAttention_49572512530497.

Sparse attention (local 256-window causal + strided-64 global, GQA 16q/4kv,
RoPE, sigmoid head gating) with fused projections, for B=2, S=2048, DIM=2048.

Sharding: 8 cores = 2 batches x 4 kv-head groups. Core c=(b,g) computes the
full pipeline for batch b and q-heads [4g, 4g+4) (which share kv head g), and
produces the partial output  attn_heads @ Wo.T[rows 512g:512(g+1)] of shape
[S, DIM].  The host sums the 4 per-group partials of each batch.

Instruction-count-oriented design (per-op floors dominate on TRN2):
 - projections / scores / output matmuls in float32r (full PE rate, fp32 data)
 - additive window mask applied by PE (identity @ mask accumulated into PSUM)
 - local + global scores share one PSUM bank; one exp each with accum_out
 - softmax normalization (and the 0.7/0.3 mix weights) folded into the
   P-transpose via a diag(w/l) moving operand built by GPSIMD
 - AV in bf16; all 4 heads accumulate into one PSUM bank per q-tile
 - RoPE multiplies on DVE (PSUM readers), adds on GPSIMD
"""

import numpy as np

import concourse.bass as bass
import concourse.mybir as mybir
import concourse.tile as tile
from concourse import bacc
from concourse.bass_utils import run_bass_kernel_spmd

B, S, DIM = 2, 2048, 2048
NH, NKV = 16, 4
HD = DIM // NH            # 128
GQ = NH // NKV            # 4 q-heads per kv head / per core
BASE = 10000.0
WINDOW, STRIDE = 256, 64
NG = S // STRIDE          # 32 global keys
SCALE = 1.0 / float(np.sqrt(HD))
NQT = S // 128            # 16 query tiles
NKC = DIM // 128          # 16 contraction chunks
NST = 4                   # seq strips for projections
STRIP = S // NST          # 512
MASKVAL = -1e30

f32 = mybir.dt.float32
f32r = mybir.dt.float32r
bf16 = mybir.dt.bfloat16
EXP = mybir.ActivationFunctionType.Exp
SIGMOID = mybir.ActivationFunctionType.Sigmoid


def _rope_tables():
    half = HD // 2
    inv_freq = 1.0 / (BASE ** (np.arange(0, half, dtype=np.float64) * 2.0 / HD))
    t = np.arange(S, dtype=np.float64)
    freqs = t[:, None] * inv_freq[None, :]          # [S, 64]
    cosT = np.cos(freqs).T.astype(np.float32)       # [64, S]
    sinT = np.sin(freqs).T.astype(np.float32)
    cos2 = np.concatenate([cosT, cosT], axis=0)     # [128, S]
    sin2s = np.concatenate([-sinT, sinT], axis=0)   # [128, S]
    return cos2, sin2s


def _win(qt):
    q0 = qt * 128
    wstart = max(0, q0 - WINDOW)
    return wstart, q0 + 128 - wstart


def _mask(qt):
    q0 = qt * 128
    wstart, w = _win(qt)
    qi = np.arange(128)[:, None] + q0
    kj = np.arange(w)[None, :] + wstart
    allowed = (kj <= qi) & (kj >= qi - WINDOW)
    return np.where(allowed, 0.0, MASKVAL).astype(np.float32)


def _build_nc():
    nc = bacc.Bacc()

    xt_d = nc.dram_tensor("xt", [DIM, S], f32r, kind="ExternalInput")
    wq_d = nc.dram_tensor("wq", [NKC, 128, GQ * 128], f32r, kind="ExternalInput")
    wkv_d = nc.dram_tensor("wkv", [NKC, 128, 256], f32r, kind="ExternalInput")
    wr_d = nc.dram_tensor("wr", [NKC, 128, GQ], f32r, kind="ExternalInput")
    br_d = nc.dram_tensor("br", [GQ, 1], f32, kind="ExternalInput")
    wo_d = nc.dram_tensor("wo", [GQ, 128, DIM], f32r, kind="ExternalInput")
    out_d = nc.dram_tensor("out", [S, DIM], f32, kind="ExternalOutput")

    cos2_np, sin2s_np = _rope_tables()
    cos2_d = nc.inline_tensor(cos2_np, "cos2c")
    sin2s_d = nc.inline_tensor(sin2s_np, "sin2sc")
    kj = np.arange(128)[:, None]
    qi = np.arange(128)[None, :]
    mlo = np.where(kj >= qi, 0.0, MASKVAL).astype(np.float32)   # first window chunk
    mhi = np.where(kj <= qi, 0.0, MASKVAL).astype(np.float32)   # diagonal chunk
    mlo_d = nc.inline_tensor(np.tile(mlo, (1, GQ)), "mloc")     # [128, 512]
    mhi_d = nc.inline_tensor(np.tile(mhi, (1, GQ)), "mhic")
    eye = np.eye(128)
    idf_d = nc.inline_tensor(eye.astype(np.float32), "idfc")
    import ml_dtypes
    w07_d = nc.inline_tensor(np.full((128, 1), 1.0 / 0.7, ml_dtypes.bfloat16),
                             "w07c")
    w03_d = nc.inline_tensor(np.full((128, 1), 1.0 / 0.3, ml_dtypes.bfloat16),
                             "w03c")
    ones1_d = nc.inline_tensor(np.ones((128, 128), np.float32), "ones1c")

    with tile.TileContext(nc) as tc:
        with tc.tile_pool(name="glob", bufs=1) as glob:
            qT = glob.tile([128, GQ * S], f32r, tag="qTa", name="qTa")
            qTh_view = qT.rearrange("p (h s) -> p h s", h=GQ)
            kT = glob.tile([128, S], f32r, tag="kT", name="kT")
            vT = glob.tile([128, S], f32, tag="vT", name="vT")
            v_bf = glob.tile([128, S], bf16, tag="v_bf", name="v_bf")
            vg_bf = glob.tile([32, 128], bf16, tag="vgbf", name="vgbf")
            kg = glob.tile([128, NG], f32r, tag="kg", name="kg")
            gateS = glob.tile([GQ, S], f32r, tag="gateS", name="gateS")
            gAB = [glob.tile([65, S], f32r, tag=f"gAB{i}", name=f"gAB{i}")
                   for i in range(2)]
            def _grow(h, sl=slice(None)):
                return gAB[h // 2][(h % 2) * 64:(h % 2) * 64 + 1, sl]
            cos2 = glob.tile([128, S], f32, tag="cos2", name="cos2")
            sin2s = glob.tile([128, S], f32, tag="sin2s", name="sin2s")
            m_lo = glob.tile([128, 512], f32r, tag="m_lo", name="m_lo")
            m_hi = glob.tile([128, 512], f32r, tag="m_hi", name="m_hi")
            id_f = glob.tile([128, 128], f32, tag="idf", name="idf")
            id_r = glob.tile([128, 128], f32r, tag="idr", name="idr")
            w07_bf = glob.tile([128, 1], bf16, tag="w07bf", name="w07bf")
            w03_bf = glob.tile([128, 1], bf16, tag="w03bf", name="w03bf")
            ones1_r = glob.tile([128, 128], f32r, tag="ones1r", name="ones1r")
            br_t = glob.tile([GQ, 1], f32, tag="br", name="br")

            nc.sync.dma_start(out=br_t, in_=br_d[:, :])
            nc.sync.dma_start(out=cos2, in_=cos2_d[:, :])
            nc.sync.dma_start(out=sin2s, in_=sin2s_d[:, :])
            nc.sync.dma_start(out=ones1_r, in_=ones1_d[:, :].bitcast(f32r))

            # ================= phase 1: fused projections =================
            with tc.tile_pool(name="wts", bufs=1) as wpool, \
                 tc.tile_pool(name="xs", bufs=12) as xpool, \
                 tc.tile_pool(name="pps", bufs=1, space="PSUM") as ppool, \
                 tc.tile_pool(name="ptmp", bufs=4) as tpool:
                wq_sb = [wpool.tile([128, GQ * 128], f32r, tag=f"wq{k}",
                                    name=f"wq{k}") for k in range(NKC)]
                wkv_sb = [wpool.tile([128, 256], f32r, tag=f"wkv{k}",
                                     name=f"wkv{k}") for k in range(NKC)]
                wr_sb = [wpool.tile([128, GQ], f32r, tag=f"wr{k}", name=f"wr{k}")
                         for k in range(NKC)]

                for st in range(NST):
                    sl = slice(st * STRIP, (st + 1) * STRIP)
                    q_ps = [ppool.tile([128, STRIP], f32, tag=f"qps{d}",
                                       name=f"qps{d}") for d in range(GQ)]
                    kv_ps = [ppool.tile([128, STRIP], f32, tag=f"kvps{d}",
                                        name=f"kvps{d}") for d in range(2)]
                    g_ps = ppool.tile([GQ, STRIP], f32, tag="gps2", name="gps", bufs=1)
                    for k in range(NKC):
                        xk = xpool.tile([128, STRIP], f32r, tag="xk", name="xk")
                        nc.sync.dma_start(
                            out=xk, in_=xt_d[k * 128:(k + 1) * 128, sl])
                        if st == 0:
                            nc.sync.dma_start(out=wq_sb[k], in_=wq_d[k])
                            nc.sync.dma_start(out=wkv_sb[k], in_=wkv_d[k])
                            nc.sync.dma_start(out=wr_sb[k], in_=wr_d[k])
                        mmargs = dict(start=(k == 0), stop=(k == NKC - 1))
                        for d in range(GQ):
                            nc.tensor.matmul(
                                q_ps[d], wq_sb[k][:, d * 128:(d + 1) * 128],
                                xk, **mmargs)
                        for d in range(2):
                            nc.tensor.matmul(
                                kv_ps[d], wkv_sb[k][:, d * 128:(d + 1) * 128],
                                xk, **mmargs)
                        nc.tensor.matmul(g_ps, wr_sb[k], xk, **mmargs)

                    # gate first: sigmoid rows, then re-base each head's row
                    # to a matmul-legal start partition via tiny SBUF DMAs
                    nc.scalar.activation(gateS[:, sl], g_ps, SIGMOID,
                                         bias=br_t, scale=1.0)
                    for h in range(GQ):
                        nc.sync.dma_start(out=_grow(h, sl),
                                          in_=gateS[h:h + 1, sl])
                    # RoPE evacuation: out = ps*cos2 + swap(ps)*[-sin; sin]
                    # then the per-query sigmoid gate is folded into q via a
                    # PE broadcast of the gate row (ones1 @ gate_row -> PSUM)
                    for h in range(GQ):
                        ps = q_ps[h]
                        qsl = qTh_view[:, h, sl]
                        a_ps = ppool.tile([128, STRIP], f32, tag="gps",
                                          name="a_ps", bufs=1)
                        base = (h % 2) * 64
                        nc.tensor.matmul(a_ps, ones1_r[base:base + 1, :],
                                         _grow(h, sl),
                                         start=True, stop=True)
                        # RoPE from PSUM (swapped-half reads are PSUM-side),
                        # gate applied last from the broadcast PSUM row
                        tmp = tpool.tile([128, STRIP], f32, tag="ropetmp",
                                         name="ropetmp")
                        nc.vector.tensor_mul(tmp[0:64], ps[64:128],
                                             sin2s[0:64, sl])
                        nc.vector.tensor_mul(tmp[64:128], ps[0:64],
                                             sin2s[64:128, sl])
                        nc.vector.tensor_mul(qsl, ps, cos2[:, sl])
                        nc.gpsimd.tensor_add(qsl, qsl, tmp)
                        nc.vector.tensor_mul(qsl, qsl, a_ps)
                    ps = kv_ps[0]
                    tmp = tpool.tile([128, STRIP], f32, tag="ropetmp",
                                     name="ropetmp")
                    nc.vector.tensor_mul(tmp[0:64], ps[64:128], sin2s[0:64, sl])
                    nc.vector.tensor_mul(tmp[64:128], ps[0:64], sin2s[64:128, sl])
                    nc.vector.tensor_mul(kT[:, sl], ps, cos2[:, sl])
                    nc.gpsimd.tensor_add(kT[:, sl], kT[:, sl], tmp)
                    nc.scalar.copy(vT[:, sl], kv_ps[1])
            # ========= phase 1b: v transposes, global k/v =========
            nc.sync.dma_start(out=id_f, in_=idf_d[:, :])
            nc.sync.dma_start(out=id_r, in_=idf_d[:, :].bitcast(f32r))
            nc.sync.dma_start(out=m_lo, in_=mlo_d[:, :].bitcast(f32r))
            nc.sync.dma_start(out=m_hi, in_=mhi_d[:, :].bitcast(f32r))
            nc.sync.dma_start(out=w07_bf, in_=w07_d[:, :])
            nc.sync.dma_start(out=w03_bf, in_=w03_d[:, :])
            with tc.tile_pool(name="vtps", bufs=2, space="PSUM") as vpp, \
                 tc.tile_pool(name="tps", bufs=2) as tp2:
                # v transposes: 4 per PSUM bank, 4 wide evacuations
                for grp in range(4):
                    vp = vpp.tile([128, 512], f32, tag="vtp", name="vtp")
                    for j in range(4):
                        c = grp * 4 + j
                        nc.tensor.transpose(vp[:, j * 128:(j + 1) * 128],
                                            vT[:, c * 128:(c + 1) * 128], id_f)
                    dst = v_bf[:, grp * 512:(grp + 1) * 512]
                    if grp % 2 == 0:
                        nc.scalar.copy(dst, vp)
                    else:
                        nc.vector.tensor_copy(dst, vp)
                # dense copies of the strided global k/v slices
                vgs = tp2.tile([128, NG], f32, tag="vgs", name="vgs")
                nc.scalar.copy(vgs, vT[:, 0:S:STRIDE])
                nc.scalar.copy(kg, kT[:, 0:S:STRIDE])
                vgp = vpp.tile([32, 128], f32, tag="vgtp", name="vgtp", bufs=1)
                nc.tensor.transpose(vgp, vgs, id_f)
                nc.scalar.copy(vg_bf, vgp)

            # ============ phase 2: attention + output projection ============
            # S^T orientation: scores come out pre-transposed, all 4 GQA heads
            # wide (N=512).  Row sums via ones-matmuls; per-query normalization
            # and the 0.7/0.3 mix applied post-AV with PE-broadcast 1/l rows.
            with tc.tile_pool(name="wow", bufs=1) as wop, \
                 tc.tile_pool(name="att", bufs=4) as apool, \
                 tc.tile_pool(name="atts", bufs=2) as spool, \
                 tc.tile_pool(name="outp", bufs=4) as opool, \
                 tc.tile_pool(name="ps_s", bufs=3, space="PSUM") as pss, \
                 tc.tile_pool(name="ps_l", bufs=1, space="PSUM") as psl, \
                 tc.tile_pool(name="ps_av", bufs=2, space="PSUM") as psav, \
                 tc.tile_pool(name="ps_wo", bufs=2, space="PSUM") as pswo:
                woT = [wop.tile([128, DIM], f32r, tag=f"wo{h}", name=f"wo{h}")
                       for h in range(GQ)]
                for h in range(GQ):
                    nc.sync.dma_start(out=woT[h], in_=wo_d[h])

                for qt in range(NQT):
                    q0 = qt * 128
                    wstart, w = _win(qt)
                    nch = w // 128
                    qrhs = qTh_view[:, :, q0:q0 + 128]        # [128, GQ, 128]
                    l_ps = psl.tile([64, 512], f32, tag="lps", name="lps")
                    # ---- local chunks: S^T, mask, exp, l, AV ----
                    av_l = psav.tile([128, 512], f32, tag="av", name="av_l")
                    pTs = []
                    for c in range(nch):
                        kc = wstart // 128 + c
                        ksl = slice(kc * 128, (kc + 1) * 128)
                        sp = pss.tile([128, 512], f32, tag="sps", name="sps")
                        last = (qt == 0) or (c == nch - 1) or (qt >= 2 and c == 0)
                        nc.tensor.matmul(sp, kT[:, ksl], qrhs,
                                         start=True, stop=not last)
                        if qt >= 2 and c == 0:
                            nc.tensor.matmul(sp, id_r, m_lo, start=False,
                                             stop=True)
                        elif c == nch - 1:
                            nc.tensor.matmul(sp, id_r, m_hi, start=False,
                                             stop=True)
                        pT = apool.tile([128, 512], bf16, tag="pT", name="pT")
                        nc.scalar.activation(pT, sp, EXP, scale=SCALE)
                        nc.tensor.matmul(l_ps[0:1, :], w07_bf, pT,
                                         start=(c == 0), stop=(c == nch - 1))
                        nc.tensor.matmul(av_l, v_bf[:, ksl], pT,
                                         start=(c == 0), stop=(c == nch - 1))
                        pTs.append(pT)
                    # ---- global: S^T_g, exp, l_g, AV_g ----
                    spg = pss.tile([32, 512], f32, tag="sps", name="spg")
                    nc.tensor.matmul(spg, kg, qrhs, start=True, stop=True)
                    pTg = apool.tile([32, 512], bf16, tag="pTg", name="pTg")
                    nc.scalar.activation(pTg, spg, EXP, scale=SCALE)
                    nc.tensor.matmul(l_ps[32:33, :], w03_bf[0:32, :], pTg,
                                     start=True, stop=True)
                    av_g = psav.tile([128, 512], f32, tag="av", name="av_g")
                    nc.tensor.matmul(av_g, vg_bf, pTg, start=True, stop=True)
                    # ---- normalization + 0.7/0.3 mix ----
                    r_l = spool.tile([1, 512], f32r, tag="r_l", name="r_l")
                    r_g = spool.tile([1, 512], f32r, tag="r_g", name="r_g")
                    with nc.allow_low_precision("f32r == f32 bits"):
                        nc.vector.reciprocal(r_l, l_ps[0:1, :])
                        nc.vector.reciprocal(r_g, l_ps[32:33, :])
                    rbp_l = pss.tile([128, 512], f32, tag="sps", name="rbp_l")
                    nc.tensor.matmul(rbp_l, ones1_r[0:1, :], r_l,
                                     start=True, stop=True)
                    rbp_g = pss.tile([128, 512], f32, tag="sps", name="rbp_g")
                    nc.tensor.matmul(rbp_g, ones1_r[0:1, :], r_g,
                                     start=True, stop=True)
                    rb_l = spool.tile([128, 512], f32, tag="rb_l", name="rb_l")
                    rb_g = spool.tile([128, 512], f32, tag="rb_g", name="rb_g")
                    nc.scalar.copy(rb_l, rbp_l)
                    nc.vector.tensor_copy(rb_g, rbp_g)
                    t_l = spool.tile([128, 512], f32, tag="t_l", name="t_l")
                    t_g = spool.tile([128, 512], f32, tag="t_g", name="t_g")
                    nc.vector.tensor_mul(t_l, av_l, rb_l)
                    nc.vector.tensor_mul(t_g, av_g, rb_g)
                    at_all = spool.tile([128, 512], f32r, tag="at", name="at", bufs=3)
                    nc.gpsimd.tensor_add(at_all, t_l, t_g)
                    # ---- output projection for this q tile ----
                    for os_ in range(4):
                        osl = slice(os_ * 512, (os_ + 1) * 512)
                        wo_ps = pswo.tile([128, 512], f32, tag="wops", name="wops")
                        for h in range(GQ):
                            nc.tensor.matmul(wo_ps,
                                             at_all[:, h * 128:(h + 1) * 128],
                                             woT[h][:, osl],
                                             start=(h == 0), stop=(h == GQ - 1))
                        ot = opool.tile([128, 512], f32, tag="ot", name="ot")
                        if os_ % 2 == 0:
                            nc.scalar.copy(ot, wo_ps)
                        else:
                            nc.vector.tensor_copy(ot, wo_ps)
                        nc.sync.dma_start(out=out_d[q0:q0 + 128, osl], in_=ot)

    nc.finalize()
    return nc


_NC_CACHE = {}


def _get_nc():
    if "nc" not in _NC_CACHE:
        _NC_CACHE["nc"] = _build_nc()
    return _NC_CACHE["nc"]


def _prep_core_inputs(x, Wq, Wkv, Wo, Wr, br, b, g):
    xt = np.ascontiguousarray(x[b].T).astype(np.float32)           # [DIM, S]
    wq_slice = Wq[g * GQ * HD:(g + 1) * GQ * HD, :]                # [512, DIM]
    wq_t = np.ascontiguousarray(
        wq_slice.T.reshape(NKC, 128, GQ * 128)).astype(np.float32)
    krow = Wkv[g * HD:(g + 1) * HD, :]                             # [128, DIM]
    vrow = Wkv[NKV * HD + g * HD: NKV * HD + (g + 1) * HD, :]      # [128, DIM]
    kv = np.concatenate([krow, vrow], axis=0)                      # [256, DIM]
    wkv_t = np.ascontiguousarray(
        kv.T.reshape(NKC, 128, 256)).astype(np.float32)
    wr_slice = Wr[g * GQ:(g + 1) * GQ, :]                          # [4, DIM]
    wr_t = np.ascontiguousarray(wr_slice.T.reshape(NKC, 128, GQ)).astype(np.float32)
    br_s = np.ascontiguousarray(
        br[g * GQ:(g + 1) * GQ].reshape(GQ, 1)).astype(np.float32)
    wo_t = np.ascontiguousarray(
        Wo[:, g * GQ * HD:(g + 1) * GQ * HD].T.reshape(GQ, 128, DIM)
    ).astype(np.float32)
    return {"xt": xt, "wq": wq_t, "wkv": wkv_t, "wr": wr_t, "br": br_s,
            "wo": wo_t}


def kernel(x, Wq, Wkv, Wo, Wr, br):
    x = np.asarray(x, dtype=np.float32)
    Wq = np.asarray(Wq, dtype=np.float32)
    Wkv = np.asarray(Wkv, dtype=np.float32)
    Wo = np.asarray(Wo, dtype=np.float32)
    Wr = np.asarray(Wr, dtype=np.float32)
    br = np.asarray(br, dtype=np.float32)

    nc = _get_nc()
    in_maps = []
    for c in range(8):
        b, g = divmod(c, NKV)
        in_maps.append(_prep_core_inputs(x, Wq, Wkv, Wo, Wr, br, b, g))
    res = run_bass_kernel_spmd(nc, in_maps, list(range(8)))
    out = np.zeros((B, S, DIM), dtype=np.float32)
    for c in range(8):
        b, g = divmod(c, NKV)
        out[b] += res.results[c]["out"]
    return out



# revision 5
# speedup vs baseline: 4.2470x; 4.2470x over previous
"""Trainium2 Bass kernel for nn_CausalSelfAttention_49572512530497.

Sparse attention (local 256-window causal + strided-64 global, GQA 16q/4kv,
RoPE, sigmoid head gating) with fused projections, for B=2, S=2048, DIM=2048.

Sharding: 8 cores = 2 batches x 4 contraction/head-group slices, with
on-device collectives to eliminate input duplication and host-side reduction:

 - Core c=(b,g) receives bf16 slices x[b][:, 512g:512g+512], Wq[:, cols],
   Wkv[:, cols], Wo[:, 512g:512g+512], Wr[:, cols].T  (7 MB/core vs 26 MB
   for head-sharding with replicated x).
 - Phase 1 computes PARTIAL q/k/v/gate projections for ALL 16 heads of
   batch b (contraction over its 512-column slice of DIM), laid out in
   DRAM as 4 head-group chunks.
 - ReduceScatter(add) over the 4 cores of each batch delivers to core
   (b,g) the COMPLETE q (4 heads), k/v (kv head g) and gate logits.
 - Phase 2 = RoPE + sigmoid gating + windowed-local+strided-global
   attention + output projection partial (same instruction structure as
   the head-sharded kernel: f32r score matmuls, PE-applied additive
   masks, shared-PSUM softmax, bf16 AV, diag(1/l) normalization).
 - A second ReduceScatter over sequence chunks gives each core a disjoint
   [512, 2048] slice of the final output: D2H is 32 MB total, no host sum.

All input tensors ship as bf16 (PE products of bf16 operands accumulate
exactly in f32, so matmul precision matches f32 compute on bf16-quantized
data); device-side PE transposes produce the [contraction, free] layouts,
so the host does no large transposes.
"""

import numpy as np
import ml_dtypes

import concourse.bass as bass
import concourse.mybir as mybir
import concourse.tile as tile
from concourse import bacc
from concourse.bass_utils import run_bass_kernel_spmd

B, S, DIM = 2, 2048, 2048
NH, NKV = 16, 4
HD = DIM // NH            # 128
GQ = NH // NKV            # 4 q-heads per kv head / per core
BASE = 10000.0
WINDOW, STRIDE = 256, 64
NG = S // STRIDE          # 32 global keys
SCALE = 1.0 / float(np.sqrt(HD))
NQT = S // 128            # 16 query tiles
CSL = DIM // 4            # 512 contraction columns per core
NCC = CSL // 128          # 4 contraction chunks
NST = 4                   # seq strips for projections
STRIP = S // NST          # 512
MASKVAL = -1e30
GROUPS = [[0, 1, 2, 3], [4, 5, 6, 7]]
# partial chunk layout (rows): q heads 4g..4g+4 | k head g | v head g | gates
PQ, PK, PV, PGT = 0, 512, 640, 768
PROWS = 772

f32 = mybir.dt.float32
f32r = mybir.dt.float32r
bf16 = mybir.dt.bfloat16
EXP = mybir.ActivationFunctionType.Exp
SIGMOID = mybir.ActivationFunctionType.Sigmoid


def _rope_tables():
    half = HD // 2
    inv_freq = 1.0 / (BASE ** (np.arange(0, half, dtype=np.float64) * 2.0 / HD))
    t = np.arange(S, dtype=np.float64)
    freqs = t[:, None] * inv_freq[None, :]          # [S, 64]
    cosT = np.cos(freqs).T.astype(np.float32)       # [64, S]
    sinT = np.sin(freqs).T.astype(np.float32)
    cos2 = np.concatenate([cosT, cosT], axis=0)     # [128, S]
    sin2s = np.concatenate([-sinT, sinT], axis=0)   # [128, S]
    return cos2, sin2s


def _win(qt):
    q0 = qt * 128
    wstart = max(0, q0 - WINDOW)
    return wstart, q0 + 128 - wstart


def _build_nc():
    nc = bacc.Bacc(num_devices=8)

    xg_d = nc.dram_tensor("xg", [16, 128, CSL], bf16, kind="ExternalInput")
    wq_d = nc.dram_tensor("wq", [16, 128, CSL], bf16, kind="ExternalInput")
    wkv_d = nc.dram_tensor("wkv", [8, 128, CSL], bf16, kind="ExternalInput")
    wo_d = nc.dram_tensor("wo", [16, 128, CSL], bf16, kind="ExternalInput")
    wrT_d = nc.dram_tensor("wrT", [NCC, 128, NH], bf16, kind="ExternalInput")
    br_d = nc.dram_tensor("br", [GQ, 1], f32, kind="ExternalInput")
    out_d = nc.dram_tensor("out", [STRIP, DIM], f32, kind="ExternalOutput")

    cos2_np, sin2s_np = _rope_tables()
    cos2_d = nc.inline_tensor(cos2_np, "cos2c")
    sin2s_d = nc.inline_tensor(sin2s_np, "sin2sc")
    kj = np.arange(128)[:, None]
    qi = np.arange(128)[None, :]
    mlo = np.where(kj >= qi, 0.0, MASKVAL).astype(np.float32)   # first window chunk
    mhi = np.where(kj <= qi, 0.0, MASKVAL).astype(np.float32)   # diagonal chunk
    mlo_d = nc.inline_tensor(np.tile(mlo, (1, GQ)), "mloc")     # [128, 512]
    mhi_d = nc.inline_tensor(np.tile(mhi, (1, GQ)), "mhic")
    eye = np.eye(128)
    idf_d = nc.inline_tensor(eye.astype(np.float32), "idfc")
    idb_d = nc.inline_tensor(eye.astype(ml_dtypes.bfloat16), "idbc")
    w07_d = nc.inline_tensor(np.full((128, 1), 1.0 / 0.7, ml_dtypes.bfloat16),
                             "w07c")
    w03_d = nc.inline_tensor(np.full((128, 1), 1.0 / 0.3, ml_dtypes.bfloat16),
                             "w03c")
    ones1_d = nc.inline_tensor(np.ones((128, 128), np.float32), "ones1c")

    with tile.TileContext(nc) as tc:
        with tc.tile_pool(name="glob", bufs=1) as glob, \
             tc.tile_pool(name="gdram", bufs=1, space="DRAM") as gdram:
            part_d = gdram.tile([GQ, PROWS, S], f32, name="part_d")
            rs_d = gdram.tile([PROWS, S], f32, name="rs_d")
            po_d = gdram.tile([GQ, STRIP, DIM], f32, name="po_d")
            ro_d = gdram.tile([STRIP, DIM], f32, name="ro_d")

            qT = glob.tile([128, GQ * S], f32r, tag="qTa", name="qTa")
            qTh_view = qT.rearrange("p (h s) -> p h s", h=GQ)
            kT = glob.tile([128, S], f32r, tag="kT", name="kT")
            vT = glob.tile([128, S], f32, tag="vT", name="vT")
            v_bf = glob.tile([128, S], bf16, tag="v_bf", name="v_bf")
            vg_bf = glob.tile([32, 128], bf16, tag="vgbf", name="vgbf")
            kg = glob.tile([128, NG], f32r, tag="kg", name="kg")
            gateS = glob.tile([GQ, S], f32r, tag="gateS", name="gateS")
            gAB = [glob.tile([65, S], f32r, tag=f"gAB{i}", name=f"gAB{i}")
                   for i in range(2)]
            def _grow(h, sl=slice(None)):
                return gAB[h // 2][(h % 2) * 64:(h % 2) * 64 + 1, sl]
            cos2 = glob.tile([128, S], f32, tag="cos2", name="cos2")
            sin2s = glob.tile([128, S], f32, tag="sin2s", name="sin2s")
            m_lo = glob.tile([128, 512], f32r, tag="m_lo", name="m_lo")
            m_hi = glob.tile([128, 512], f32r, tag="m_hi", name="m_hi")
            id_f = glob.tile([128, 128], f32, tag="idf", name="idf")
            id_r = glob.tile([128, 128], f32r, tag="idr", name="idr")
            id_b = glob.tile([128, 128], bf16, tag="idb", name="idb")
            w07_bf = glob.tile([128, 1], bf16, tag="w07bf", name="w07bf")
            w03_bf = glob.tile([128, 1], bf16, tag="w03bf", name="w03bf")
            ones1_r = glob.tile([128, 128], f32r, tag="ones1r", name="ones1r")
            br_t = glob.tile([GQ, 1], f32, tag="br", name="br")
            woT = [glob.tile([128, DIM], bf16, tag=f"wo{h}", name=f"wo{h}")
                   for h in range(GQ)]

            nc.sync.dma_start(out=br_t, in_=br_d[:, :])
            nc.sync.dma_start(out=cos2, in_=cos2_d[:, :])
            nc.sync.dma_start(out=sin2s, in_=sin2s_d[:, :])
            nc.sync.dma_start(out=ones1_r, in_=ones1_d[:, :].bitcast(f32r))
            nc.sync.dma_start(out=id_f, in_=idf_d[:, :])
            nc.sync.dma_start(out=id_r, in_=idf_d[:, :].bitcast(f32r))
            nc.sync.dma_start(out=id_b, in_=idb_d[:, :])
            nc.sync.dma_start(out=m_lo, in_=mlo_d[:, :].bitcast(f32r))
            nc.sync.dma_start(out=m_hi, in_=mhi_d[:, :].bitcast(f32r))
            nc.sync.dma_start(out=w07_bf, in_=w07_d[:, :])
            nc.sync.dma_start(out=w03_bf, in_=w03_d[:, :])

            # ===== phase 0: stage bf16 inputs, PE-transpose to [contr, free] =====
            # xT[cc]   [128, S]    : x[b][:, cols].T chunk cc
            # wqT[cc]  [128, 2048] : Wq[:, cols].T   (free = q out-dims, 16 heads)
            # wkvT[cc] [128, 1024] : Wkv[:, cols].T  (free = k heads | v heads)
            # woT[h]   [128, 2048] : Wo[:, 512g+128h ...].T (free = out-dims)
            with tc.tile_pool(name="ph1", bufs=1) as ph1:
                xT = [ph1.tile([128, S], bf16, tag=f"xT{c}", name=f"xT{c}")
                      for c in range(NCC)]
                wqT = [ph1.tile([128, DIM], bf16, tag=f"wqT{c}", name=f"wqT{c}")
                       for c in range(NCC)]
                wkvT = [ph1.tile([128, 1024], bf16, tag=f"wkvT{c}",
                                 name=f"wkvT{c}") for c in range(NCC)]
                wr_sb = [ph1.tile([128, NH], bf16, tag=f"wr{c}", name=f"wr{c}")
                         for c in range(NCC)]
                for c in range(NCC):
                    nc.sync.dma_start(out=wr_sb[c], in_=wrT_d[c])

                with tc.tile_pool(name="stage", bufs=2) as stage, \
                     tc.tile_pool(name="tps", bufs=4, space="PSUM") as tps:
                    eng = [nc.scalar.copy, nc.vector.tensor_copy]
                    ei = 0
                    for src_d, nb, dsts in ((xg_d, 16, xT), (wq_d, 16, wqT),
                                            (wkv_d, 8, wkvT), (wo_d, 16, woT)):
                        st_t = stage.tile([128, nb * CSL], bf16, tag="stg",
                                          name="stg")
                        for sb in range(nb):
                            nc.sync.dma_start(
                                out=st_t[:, sb * CSL:(sb + 1) * CSL],
                                in_=src_d[sb])
                        for cc in range(NCC):
                            if dsts is woT:
                                dst = woT[cc]          # head h = hd-chunk cc
                            else:
                                dst = dsts[cc]
                            for grp in range(nb // 4):
                                pt = tps.tile([128, 512], bf16, tag="tp",
                                              name="tp")
                                for j in range(4):
                                    sb = grp * 4 + j
                                    nc.tensor.transpose(
                                        pt[:, j * 128:(j + 1) * 128],
                                        st_t[:, sb * CSL + cc * 128:
                                             sb * CSL + cc * 128 + 128],
                                        id_b)
                                eng[ei % 2](dst[:, grp * 512:(grp + 1) * 512],
                                            pt)
                                ei += 1

                # ===== phase 1: partial projections for all 16 heads =====
                with tc.tile_pool(name="pps", bufs=4, space="PSUM") as ppool, \
                     tc.tile_pool(name="gps", bufs=2, space="PSUM") as gpool, \
                     tc.tile_pool(name="pev", bufs=6) as epool:
                    ei = 0
                    for st in range(NST):
                        sl = slice(st * STRIP, (st + 1) * STRIP)
                        # q blocks (head h) and kv blocks
                        for ob in range(24):
                            pp = ppool.tile([128, STRIP], f32, tag="pp",
                                            name="pp")
                            for cc in range(NCC):
                                if ob < 16:
                                    stat = wqT[cc][:, ob * 128:(ob + 1) * 128]
                                else:
                                    kb = ob - 16
                                    stat = wkvT[cc][:, kb * 128:(kb + 1) * 128]
                                nc.tensor.matmul(pp, stat, xT[cc][:, sl],
                                                 start=(cc == 0),
                                                 stop=(cc == NCC - 1))
                            ev = epool.tile([128, STRIP], f32, tag="ev",
                                            name="ev")
                            eng[ei % 2](ev, pp)
                            ei += 1
                            if ob < 16:          # q head ob
                                gg, r0 = ob // GQ, PQ + (ob % GQ) * 128
                            elif ob < 20:        # k head ob-16
                                gg, r0 = ob - 16, PK
                            else:                # v head ob-20
                                gg, r0 = ob - 20, PV
                            nc.sync.dma_start(
                                out=part_d[gg, r0:r0 + 128, sl], in_=ev)
                        # gate logits for all 16 heads
                        gp = gpool.tile([NH, STRIP], f32, tag="gp", name="gp")
                        for cc in range(NCC):
                            nc.tensor.matmul(gp, wr_sb[cc], xT[cc][:, sl],
                                             start=(cc == 0),
                                             stop=(cc == NCC - 1))
                        gev = epool.tile([NH, STRIP], f32, tag="gev",
                                         name="gev")
                        nc.vector.tensor_copy(gev, gp)
                        for gg in range(GQ):
                            nc.sync.dma_start(
                                out=part_d[gg, PGT:PGT + GQ, sl],
                                in_=gev[gg * GQ:(gg + 1) * GQ, :])

            # ===== ReduceScatter #1: complete q/k/v/gate for own head group ====
            nc.gpsimd.collective_compute(
                "ReduceScatter", mybir.AluOpType.add, replica_groups=GROUPS,
                ins=[part_d.opt()], outs=[rs_d.opt()])

            for h in range(GQ):
                nc.sync.dma_start(
                    out=qTh_view[:, h, :],
                    in_=rs_d[PQ + h * 128:PQ + (h + 1) * 128, :].bitcast(f32r))
            nc.sync.dma_start(out=kT, in_=rs_d[PK:PK + 128, :].bitcast(f32r))
            nc.sync.dma_start(out=vT, in_=rs_d[PV:PV + 128, :])
            glogit = glob.tile([GQ, S], f32, tag="glogit", name="glogit")
            nc.sync.dma_start(out=glogit, in_=rs_d[PGT:PGT + GQ, :])

            # ===== phase 1.5: sigmoid gate, RoPE, v transposes, global k/v ====
            with tc.tile_pool(name="rtmp", bufs=3) as rpool, \
                 tc.tile_pool(name="aps", bufs=2, space="PSUM") as apsp, \
                 tc.tile_pool(name="vtps", bufs=2, space="PSUM") as vpp:
                nc.scalar.activation(gateS, glogit, SIGMOID, bias=br_t,
                                     scale=1.0)
                for h in range(GQ):
                    nc.sync.dma_start(out=_grow(h), in_=gateS[h:h + 1, :])
                # RoPE k (in place): k = k*cos + swap(k)*[-sin; sin].
                # The swapped-half copy is re-read from rs_d with halves
                # exchanged so every DVE operand pair is partition-aligned.
                swp = rpool.tile([128, S], f32, tag="swp", name="swp")
                nc.sync.dma_start(out=swp[0:64], in_=rs_d[PK + 64:PK + 128, :])
                nc.sync.dma_start(out=swp[64:128], in_=rs_d[PK:PK + 64, :])
                tmp = rpool.tile([128, S], f32, tag="ropetmp", name="ropetmp")
                nc.vector.tensor_mul(tmp, swp, sin2s)
                nc.vector.tensor_mul(kT, kT, cos2)
                nc.gpsimd.tensor_add(kT, kT, tmp)
                # RoPE q + sigmoid gate fold (PE-broadcast gate rows)
                for h in range(GQ):
                    qsl = qTh_view[:, h, :]
                    r0 = PQ + h * 128
                    swp = rpool.tile([128, S], f32, tag="swp", name="swp")
                    nc.sync.dma_start(out=swp[0:64],
                                      in_=rs_d[r0 + 64:r0 + 128, :])
                    nc.sync.dma_start(out=swp[64:128],
                                      in_=rs_d[r0:r0 + 64, :])
                    tmp = rpool.tile([128, S], f32, tag="ropetmp",
                                     name="ropetmp")
                    nc.vector.tensor_mul(tmp, swp, sin2s)
                    nc.vector.tensor_mul(qsl, qsl, cos2)
                    nc.gpsimd.tensor_add(qsl, qsl, tmp)
                    base = (h % 2) * 64
                    for st in range(NST):
                        sl = slice(st * STRIP, (st + 1) * STRIP)
                        a_ps = apsp.tile([128, STRIP], f32, tag="aps",
                                         name="a_ps")
                        nc.tensor.matmul(a_ps, ones1_r[base:base + 1, :],
                                         _grow(h, sl), start=True, stop=True)
                        nc.vector.tensor_mul(qTh_view[:, h, sl],
                                             qTh_view[:, h, sl], a_ps)
                # v transposes: 4 per PSUM bank, 4 wide evacuations
                for grp in range(4):
                    vp = vpp.tile([128, 512], f32, tag="vtp", name="vtp")
                    for j in range(4):
                        c = grp * 4 + j
                        nc.tensor.transpose(vp[:, j * 128:(j + 1) * 128],
                                            vT[:, c * 128:(c + 1) * 128], id_f)
                    dst = v_bf[:, grp * 512:(grp + 1) * 512]
                    if grp % 2 == 0:
                        nc.scalar.copy(dst, vp)
                    else:
                        nc.vector.tensor_copy(dst, vp)
                # dense copies of the strided global k/v slices
                vgs = rpool.tile([128, NG], f32, tag="vgs", name="vgs")
                nc.scalar.copy(vgs, vT[:, 0:S:STRIDE])
                nc.scalar.copy(kg, kT[:, 0:S:STRIDE])
                vgp = vpp.tile([32, 128], f32, tag="vgtp", name="vgtp", bufs=1)
                nc.tensor.transpose(vgp, vgs, id_f)
                nc.scalar.copy(vg_bf, vgp)

            # ============ phase 2: attention + output projection ============
            with tc.tile_pool(name="att", bufs=4) as apool, \
                 tc.tile_pool(name="atts", bufs=2) as spool, \
                 tc.tile_pool(name="outp", bufs=4) as opool, \
                 tc.tile_pool(name="ps_s", bufs=3, space="PSUM") as pss, \
                 tc.tile_pool(name="ps_l", bufs=1, space="PSUM") as psl, \
                 tc.tile_pool(name="ps_av", bufs=2, space="PSUM") as psav, \
                 tc.tile_pool(name="ps_wo", bufs=2, space="PSUM") as pswo:
                for qt in range(NQT):
                    q0 = qt * 128
                    wstart, w = _win(qt)
                    nch = w // 128
                    qrhs = qTh_view[:, :, q0:q0 + 128]        # [128, GQ, 128]
                    l_ps = psl.tile([64, 512], f32, tag="lps", name="lps")
                    # ---- local chunks: S^T, mask, exp, l, AV ----
                    av_l = psav.tile([128, 512], f32, tag="av", name="av_l")
                    for c in range(nch):
                        kc = wstart // 128 + c
                        ksl = slice(kc * 128, (kc + 1) * 128)
                        sp = pss.tile([128, 512], f32, tag="sps", name="sps")
                        last = (qt == 0) or (c == nch - 1) or (qt >= 2 and c == 0)
                        nc.tensor.matmul(sp, kT[:, ksl], qrhs,
                                         start=True, stop=not last)
                        if qt >= 2 and c == 0:
                            nc.tensor.matmul(sp, id_r, m_lo, start=False,
                                             stop=True)
                        elif c == nch - 1:
                            nc.tensor.matmul(sp, id_r, m_hi, start=False,
                                             stop=True)
                        pT = apool.tile([128, 512], bf16, tag="pT", name="pT")
                        nc.scalar.activation(pT, sp, EXP, scale=SCALE)
                        nc.tensor.matmul(l_ps[0:1, :], w07_bf, pT,
                                         start=(c == 0), stop=(c == nch - 1))
                        nc.tensor.matmul(av_l, v_bf[:, ksl], pT,
                                         start=(c == 0), stop=(c == nch - 1))
                    # ---- global: S^T_g, exp, l_g, AV_g ----
                    spg = pss.tile([32, 512], f32, tag="sps", name="spg")
                    nc.tensor.matmul(spg, kg, qrhs, start=True, stop=True)
                    pTg = apool.tile([32, 512], bf16, tag="pTg", name="pTg")
                    nc.scalar.activation(pTg, spg, EXP, scale=SCALE)
                    nc.tensor.matmul(l_ps[32:33, :], w03_bf[0:32, :], pTg,
                                     start=True, stop=True)
                    av_g = psav.tile([128, 512], f32, tag="av", name="av_g")
                    nc.tensor.matmul(av_g, vg_bf, pTg, start=True, stop=True)
                    # ---- normalization + 0.7/0.3 mix ----
                    r_l = spool.tile([1, 512], f32r, tag="r_l", name="r_l")
                    r_g = spool.tile([1, 512], f32r, tag="r_g", name="r_g")
                    with nc.allow_low_precision("f32r == f32 bits"):
                        nc.vector.reciprocal(r_l, l_ps[0:1, :])
                        nc.vector.reciprocal(r_g, l_ps[32:33, :])
                    rbp_l = pss.tile([128, 512], f32, tag="sps", name="rbp_l")
                    nc.tensor.matmul(rbp_l, ones1_r[0:1, :], r_l,
                                     start=True, stop=True)
                    rbp_g = pss.tile([128, 512], f32, tag="sps", name="rbp_g")
                    nc.tensor.matmul(rbp_g, ones1_r[0:1, :], r_g,
                                     start=True, stop=True)
                    rb_l = spool.tile([128, 512], f32, tag="rb_l", name="rb_l")
                    rb_g = spool.tile([128, 512], f32, tag="rb_g", name="rb_g")
                    nc.scalar.copy(rb_l, rbp_l)
                    nc.vector.tensor_copy(rb_g, rbp_g)
                    t_l = spool.tile([128, 512], f32, tag="t_l", name="t_l")
                    t_g = spool.tile([128, 512], f32, tag="t_g", name="t_g")
                    nc.vector.tensor_mul(t_l, av_l, rb_l)
                    nc.vector.tensor_mul(t_g, av_g, rb_g)
                    at_all = spool.tile([128, 512], bf16, tag="at", name="at",
                                        bufs=3)
                    nc.gpsimd.tensor_add(at_all, t_l, t_g)
                    # ---- output projection partial for this q tile ----
                    for os_ in range(4):
                        osl = slice(os_ * 512, (os_ + 1) * 512)
                        wo_ps = pswo.tile([128, 512], f32, tag="wops",
                                          name="wops")
                        for h in range(GQ):
                            nc.tensor.matmul(wo_ps,
                                             at_all[:, h * 128:(h + 1) * 128],
                                             woT[h][:, osl],
                                             start=(h == 0), stop=(h == GQ - 1))
                        ot = opool.tile([128, 512], f32, tag="ot", name="ot")
                        if os_ % 2 == 0:
                            nc.scalar.copy(ot, wo_ps)
                        else:
                            nc.vector.tensor_copy(ot, wo_ps)
                        nc.sync.dma_start(
                            out=po_d[qt // 4, (qt % 4) * 128:(qt % 4) * 128 + 128,
                                     osl],
                            in_=ot)

            # ===== ReduceScatter #2: sum head groups, scatter over seq =====
            nc.gpsimd.collective_compute(
                "ReduceScatter", mybir.AluOpType.add, replica_groups=GROUPS,
                ins=[po_d.opt()], outs=[ro_d.opt()])
            nc.sync.dma_start(out=out_d[:, :], in_=ro_d[:, :])

    nc.finalize()
    return nc


_NC_CACHE = {}


def _get_nc():
    if "nc" not in _NC_CACHE:
        _NC_CACHE["nc"] = _build_nc()
    return _NC_CACHE["nc"]


def kernel(x, Wq, Wkv, Wo, Wr, br):
    bf = ml_dtypes.bfloat16
    x = np.asarray(x)
    xb = x.astype(bf)                              # [2, 2048, 2048]
    wqb = np.asarray(Wq).astype(bf)
    wkvb = np.asarray(Wkv).astype(bf)
    wob = np.asarray(Wo).astype(bf)
    Wr = np.asarray(Wr, dtype=np.float32)
    br = np.asarray(br, dtype=np.float32)

    nc = _get_nc()
    # per-group (g) weight slices are shared by the two batches
    gmaps = []
    for g in range(4):
        cols = slice(g * CSL, (g + 1) * CSL)
        gmaps.append({
            "wq": np.ascontiguousarray(wqb[:, cols]).reshape(16, 128, CSL),
            "wkv": np.ascontiguousarray(wkvb[:, cols]).reshape(8, 128, CSL),
            "wo": np.ascontiguousarray(wob[:, cols]).reshape(16, 128, CSL),
            "wrT": np.ascontiguousarray(Wr[:, cols].T.astype(bf)
                                        ).reshape(NCC, 128, NH),
            "br": np.ascontiguousarray(
                br[g * GQ:(g + 1) * GQ].reshape(GQ, 1)),
        })
    in_maps = []
    for c in range(8):
        b, g = divmod(c, 4)
        cols = slice(g * CSL, (g + 1) * CSL)
        m = dict(gmaps[g])
        m["xg"] = np.ascontiguousarray(xb[b][:, cols]).reshape(16, 128, CSL)
        in_maps.append(m)
    res = run_bass_kernel_spmd(nc, in_maps, list(range(8)))
    out = np.empty((B, S, DIM), dtype=np.float32)
    for c in range(8):
        b, g = divmod(c, 4)
        out[b, g * STRIP:(g + 1) * STRIP, :] = res.results[c]["out"]
    return out


# revision 9
# speedup vs baseline: 4.7442x; 1.1171x over previous
"""Trainium2 Bass kernel for nn_CausalSelfAttention_49572512530497.

Sparse attention (local 256-window causal + strided-64 global, GQA 16q/4kv,
RoPE, sigmoid head gating) with fused projections, for B=2, S=2048, DIM=2048.

Sharding: 8 cores = 2 batches x 4 contraction/head-group slices, with
on-device collectives to eliminate input duplication and host-side reduction:

 - Core c=(b,g) receives bf16 slices x[b][:, 512g:512g+512], Wq[:, cols],
   Wkv[:, cols], Wo[:, 512g:512g+512], Wr[:, cols].T  (7 MB/core vs 26 MB
   for head-sharding with replicated x).
 - Phase 1 computes PARTIAL q/k/v/gate projections for ALL 16 heads of
   batch b (contraction over its 512-column slice of DIM), laid out in
   DRAM as 4 head-group chunks.
 - ReduceScatter(add) over the 4 cores of each batch delivers to core
   (b,g) the COMPLETE q (4 heads), k/v (kv head g) and gate logits.
 - Phase 2 = RoPE + sigmoid gating + windowed-local+strided-global
   attention + output projection partial (same instruction structure as
   the head-sharded kernel: f32r score matmuls, PE-applied additive
   masks, shared-PSUM softmax, bf16 AV, diag(1/l) normalization).
 - A second ReduceScatter over sequence chunks gives each core a disjoint
   [512, 2048] slice of the final output: D2H is 32 MB total, no host sum.

All input tensors ship as bf16 (PE products of bf16 operands accumulate
exactly in f32, so matmul precision matches f32 compute on bf16-quantized
data); device-side PE transposes produce the [contraction, free] layouts,
so the host does no large transposes.
"""

import numpy as np
import ml_dtypes

import concourse.bass as bass
import concourse.mybir as mybir
import concourse.tile as tile
from concourse import bacc
from concourse.bass_utils import run_bass_kernel_spmd

B, S, DIM = 2, 2048, 2048
NH, NKV = 16, 4
HD = DIM // NH            # 128
GQ = NH // NKV            # 4 q-heads per kv head / per core
BASE = 10000.0
WINDOW, STRIDE = 256, 64
NG = S // STRIDE          # 32 global keys
SCALE = 1.0 / float(np.sqrt(HD))
NQT = S // 128            # 16 query tiles
CSL = DIM // 4            # 512 contraction columns per core
NCC = CSL // 128          # 4 contraction chunks
NST = 4                   # seq strips for projections
STRIP = S // NST          # 512
MASKVAL = -1e30
GROUPS = [[0, 1, 2, 3], [4, 5, 6, 7]]
# partial chunk layout (rows): q heads 4g..4g+4 | k head g | v head g | gates
PQ, PK, PV, PGT = 0, 512, 640, 768
PROWS = 772

f32 = mybir.dt.float32
f32r = mybir.dt.float32r
bf16 = mybir.dt.bfloat16
EXP = mybir.ActivationFunctionType.Exp
SIGMOID = mybir.ActivationFunctionType.Sigmoid


def _rope_tables():
    half = HD // 2
    inv_freq = 1.0 / (BASE ** (np.arange(0, half, dtype=np.float64) * 2.0 / HD))
    t = np.arange(S, dtype=np.float64)
    freqs = t[:, None] * inv_freq[None, :]          # [S, 64]
    cosT = np.cos(freqs).T.astype(np.float32)       # [64, S]
    sinT = np.sin(freqs).T.astype(np.float32)
    cos2 = np.concatenate([cosT, cosT], axis=0)     # [128, S]
    sin2s = np.concatenate([-sinT, sinT], axis=0)   # [128, S]
    return cos2, sin2s


def _win(qt):
    q0 = qt * 128
    wstart = max(0, q0 - WINDOW)
    return wstart, q0 + 128 - wstart


def _build_nc():
    nc = bacc.Bacc(num_devices=8)

    xg_d = nc.dram_tensor("xg", [16, 128, CSL], bf16, kind="ExternalInput")
    wq_d = nc.dram_tensor("wq", [16, 128, CSL], bf16, kind="ExternalInput")
    wkv_d = nc.dram_tensor("wkv", [8, 128, CSL], bf16, kind="ExternalInput")
    wo_d = nc.dram_tensor("wo", [16, 128, CSL], bf16, kind="ExternalInput")
    wrT_d = nc.dram_tensor("wrT", [NCC, 128, NH], bf16, kind="ExternalInput")
    br_d = nc.dram_tensor("br", [GQ, 1], f32, kind="ExternalInput")
    out_d = nc.dram_tensor("out", [STRIP, DIM], bf16, kind="ExternalOutput")

    cos2_np, sin2s_np = _rope_tables()
    cos2_d = nc.inline_tensor(cos2_np, "cos2c")
    sin2s_d = nc.inline_tensor(sin2s_np, "sin2sc")
    kj = np.arange(128)[:, None]
    qi = np.arange(128)[None, :]
    mlo = np.where(kj >= qi, 0.0, MASKVAL).astype(np.float32)   # first window chunk
    mhi = np.where(kj <= qi, 0.0, MASKVAL).astype(np.float32)   # diagonal chunk
    mlo_d = nc.inline_tensor(np.tile(mlo, (1, GQ)), "mloc")     # [128, 512]
    mhi_d = nc.inline_tensor(np.tile(mhi, (1, GQ)), "mhic")
    eye = np.eye(128)
    idf_d = nc.inline_tensor(eye.astype(np.float32), "idfc")
    idb_d = nc.inline_tensor(eye.astype(ml_dtypes.bfloat16), "idbc")
    w07_d = nc.inline_tensor(np.full((128, 1), 1.0 / 0.7, ml_dtypes.bfloat16),
                             "w07c")
    w03_d = nc.inline_tensor(np.full((128, 1), 1.0 / 0.3, ml_dtypes.bfloat16),
                             "w03c")
    ones1_d = nc.inline_tensor(np.ones((128, 128), np.float32), "ones1c")

    with tile.TileContext(nc) as tc:
        with tc.tile_pool(name="glob", bufs=1) as glob, \
             tc.tile_pool(name="gdram", bufs=1, space="DRAM") as gdram:
            part_d = gdram.tile([GQ, PROWS, S], f32, name="part_d")
            rs_d = gdram.tile([PROWS, S], f32, name="rs_d")
            po_d = gdram.tile([GQ, STRIP, DIM], bf16, name="po_d")
            ro_d = gdram.tile([STRIP, DIM], bf16, name="ro_d")

            qT = glob.tile([128, GQ * S], f32r, tag="qTa", name="qTa")
            qTh_view = qT.rearrange("p (h s) -> p h s", h=GQ)
            kT = glob.tile([128, S], f32r, tag="kT", name="kT")
            vT = glob.tile([128, S], f32, tag="vT", name="vT")
            v_bf = glob.tile([128, S], bf16, tag="v_bf", name="v_bf")
            vg_bf = glob.tile([32, 128], bf16, tag="vgbf", name="vgbf")
            kg = glob.tile([128, NG], f32r, tag="kg", name="kg")
            gateS = glob.tile([GQ, S], f32r, tag="gateS", name="gateS")
            gAB = [glob.tile([65, S], f32r, tag=f"gAB{i}", name=f"gAB{i}")
                   for i in range(2)]
            def _grow(h, sl=slice(None)):
                return gAB[h // 2][(h % 2) * 64:(h % 2) * 64 + 1, sl]
            cos2 = glob.tile([128, S], f32, tag="cos2", name="cos2")
            sin2s = glob.tile([128, S], f32, tag="sin2s", name="sin2s")
            m_lo = glob.tile([128, 512], f32r, tag="m_lo", name="m_lo")
            m_hi = glob.tile([128, 512], f32r, tag="m_hi", name="m_hi")
            id_f = glob.tile([128, 128], f32, tag="idf", name="idf")
            id_r = glob.tile([128, 128], f32r, tag="idr", name="idr")
            id_b = glob.tile([128, 128], bf16, tag="idb", name="idb")
            w07_bf = glob.tile([128, 1], bf16, tag="w07bf", name="w07bf")
            w03_bf = glob.tile([128, 1], bf16, tag="w03bf", name="w03bf")
            ones1_r = glob.tile([128, 128], f32r, tag="ones1r", name="ones1r")
            br_t = glob.tile([GQ, 1], f32, tag="br", name="br")
            woT = [glob.tile([128, DIM], bf16, tag=f"wo{h}", name=f"wo{h}")
                   for h in range(GQ)]

            nc.sync.dma_start(out=br_t, in_=br_d[:, :])
            nc.sync.dma_start(out=cos2, in_=cos2_d[:, :])
            nc.sync.dma_start(out=sin2s, in_=sin2s_d[:, :])
            nc.sync.dma_start(out=ones1_r, in_=ones1_d[:, :].bitcast(f32r))
            nc.sync.dma_start(out=id_f, in_=idf_d[:, :])
            nc.sync.dma_start(out=id_r, in_=idf_d[:, :].bitcast(f32r))
            nc.sync.dma_start(out=id_b, in_=idb_d[:, :])
            nc.sync.dma_start(out=m_lo, in_=mlo_d[:, :].bitcast(f32r))
            nc.sync.dma_start(out=m_hi, in_=mhi_d[:, :].bitcast(f32r))
            nc.sync.dma_start(out=w07_bf, in_=w07_d[:, :])
            nc.sync.dma_start(out=w03_bf, in_=w03_d[:, :])

            # ===== phase 0: stage bf16 inputs, PE-transpose to [contr, free] =====
            # xT[cc]   [128, S]    : x[b][:, cols].T chunk cc
            # wqT[cc]  [128, 2048] : Wq[:, cols].T   (free = q out-dims, 16 heads)
            # wkvT[cc] [128, 1024] : Wkv[:, cols].T  (free = k heads | v heads)
            # woT[h]   [128, 2048] : Wo[:, 512g+128h ...].T (free = out-dims)
            with tc.tile_pool(name="ph1", bufs=1) as ph1:
                xT = [ph1.tile([128, S], bf16, tag=f"xT{c}", name=f"xT{c}")
                      for c in range(NCC)]
                wqT = [ph1.tile([128, DIM], bf16, tag=f"wqT{c}", name=f"wqT{c}")
                       for c in range(NCC)]
                wkvT = [ph1.tile([128, 1024], bf16, tag=f"wkvT{c}",
                                 name=f"wkvT{c}") for c in range(NCC)]
                wr_sb = [ph1.tile([128, NH], bf16, tag=f"wr{c}", name=f"wr{c}")
                         for c in range(NCC)]
                for c in range(NCC):
                    nc.sync.dma_start(out=wr_sb[c], in_=wrT_d[c])

                with tc.tile_pool(name="stage", bufs=2) as stage, \
                     tc.tile_pool(name="tps", bufs=4, space="PSUM") as tps:
                    eng = [nc.scalar.copy, nc.vector.tensor_copy]
                    ei = 0
                    for src_d, nb, dsts in ((xg_d, 16, xT), (wq_d, 16, wqT),
                                            (wkv_d, 8, wkvT), (wo_d, 16, woT)):
                        st_t = stage.tile([128, nb * CSL], bf16, tag="stg",
                                          name="stg")
                        for sb in range(nb):
                            nc.sync.dma_start(
                                out=st_t[:, sb * CSL:(sb + 1) * CSL],
                                in_=src_d[sb])
                        for cc in range(NCC):
                            if dsts is woT:
                                dst = woT[cc]          # head h = hd-chunk cc
                            else:
                                dst = dsts[cc]
                            for grp in range(nb // 4):
                                pt = tps.tile([128, 512], bf16, tag="tp",
                                              name="tp")
                                for j in range(4):
                                    sb = grp * 4 + j
                                    nc.tensor.transpose(
                                        pt[:, j * 128:(j + 1) * 128],
                                        st_t[:, sb * CSL + cc * 128:
                                             sb * CSL + cc * 128 + 128],
                                        id_b)
                                eng[ei % 2](dst[:, grp * 512:(grp + 1) * 512],
                                            pt)
                                ei += 1

                # ===== phase 1: partial projections for all 16 heads =====
                with tc.tile_pool(name="pps", bufs=4, space="PSUM") as ppool, \
                     tc.tile_pool(name="gps", bufs=2, space="PSUM") as gpool, \
                     tc.tile_pool(name="pev", bufs=6) as epool:
                    ei = 0
                    for st in range(NST):
                        sl = slice(st * STRIP, (st + 1) * STRIP)
                        # q blocks (head h) and kv blocks
                        for ob in range(24):
                            pp = ppool.tile([128, STRIP], f32, tag="pp",
                                            name="pp")
                            for cc in range(NCC):
                                if ob < 16:
                                    stat = wqT[cc][:, ob * 128:(ob + 1) * 128]
                                else:
                                    kb = ob - 16
                                    stat = wkvT[cc][:, kb * 128:(kb + 1) * 128]
                                nc.tensor.matmul(pp, stat, xT[cc][:, sl],
                                                 start=(cc == 0),
                                                 stop=(cc == NCC - 1))
                            ev = epool.tile([128, STRIP], f32, tag="ev",
                                            name="ev")
                            eng[ei % 2](ev, pp)
                            ei += 1
                            if ob < 16:          # q head ob
                                gg, r0 = ob // GQ, PQ + (ob % GQ) * 128
                            elif ob < 20:        # k head ob-16
                                gg, r0 = ob - 16, PK
                            else:                # v head ob-20
                                gg, r0 = ob - 20, PV
                            nc.sync.dma_start(
                                out=part_d[gg, r0:r0 + 128, sl], in_=ev)
                        # gate logits for all 16 heads
                        gp = gpool.tile([NH, STRIP], f32, tag="gp", name="gp")
                        for cc in range(NCC):
                            nc.tensor.matmul(gp, wr_sb[cc], xT[cc][:, sl],
                                             start=(cc == 0),
                                             stop=(cc == NCC - 1))
                        gev = epool.tile([NH, STRIP], f32, tag="gev",
                                         name="gev")
                        nc.vector.tensor_copy(gev, gp)
                        for gg in range(GQ):
                            nc.sync.dma_start(
                                out=part_d[gg, PGT:PGT + GQ, sl],
                                in_=gev[gg * GQ:(gg + 1) * GQ, :])

            # ===== ReduceScatter #1: complete q/k/v/gate for own head group ====
            nc.gpsimd.collective_compute(
                "ReduceScatter", mybir.AluOpType.add, replica_groups=GROUPS,
                ins=[part_d.opt()], outs=[rs_d.opt()])

            for h in range(GQ):
                nc.sync.dma_start(
                    out=qTh_view[:, h, :],
                    in_=rs_d[PQ + h * 128:PQ + (h + 1) * 128, :].bitcast(f32r))
            nc.sync.dma_start(out=kT, in_=rs_d[PK:PK + 128, :].bitcast(f32r))
            nc.sync.dma_start(out=vT, in_=rs_d[PV:PV + 128, :])
            glogit = glob.tile([GQ, S], f32, tag="glogit", name="glogit")
            nc.sync.dma_start(out=glogit, in_=rs_d[PGT:PGT + GQ, :])

            # ===== phase 1.5: sigmoid gate, RoPE, v transposes, global k/v ====
            with tc.tile_pool(name="rtmp", bufs=3) as rpool, \
                 tc.tile_pool(name="aps", bufs=2, space="PSUM") as apsp, \
                 tc.tile_pool(name="vtps", bufs=2, space="PSUM") as vpp:
                nc.scalar.activation(gateS, glogit, SIGMOID, bias=br_t,
                                     scale=1.0)
                for h in range(GQ):
                    nc.sync.dma_start(out=_grow(h), in_=gateS[h:h + 1, :])
                # RoPE k (in place): k = k*cos + swap(k)*[-sin; sin].
                # The swapped-half copy is re-read from rs_d with halves
                # exchanged so every DVE operand pair is partition-aligned.
                swp = rpool.tile([128, S], f32, tag="swp", name="swp")
                nc.sync.dma_start(out=swp[0:64], in_=rs_d[PK + 64:PK + 128, :])
                nc.sync.dma_start(out=swp[64:128], in_=rs_d[PK:PK + 64, :])
                tmp = rpool.tile([128, S], f32, tag="ropetmp", name="ropetmp")
                nc.vector.tensor_mul(tmp, swp, sin2s)
                nc.vector.tensor_mul(kT, kT, cos2)
                nc.gpsimd.tensor_add(kT, kT, tmp)
                # RoPE q + sigmoid gate fold (PE-broadcast gate rows)
                for h in range(GQ):
                    qsl = qTh_view[:, h, :]
                    r0 = PQ + h * 128
                    swp = rpool.tile([128, S], f32, tag="swp", name="swp")
                    nc.sync.dma_start(out=swp[0:64],
                                      in_=rs_d[r0 + 64:r0 + 128, :])
                    nc.sync.dma_start(out=swp[64:128],
                                      in_=rs_d[r0:r0 + 64, :])
                    tmp = rpool.tile([128, S], f32, tag="ropetmp",
                                     name="ropetmp")
                    nc.vector.tensor_mul(tmp, swp, sin2s)
                    nc.vector.tensor_mul(qsl, qsl, cos2)
                    nc.gpsimd.tensor_add(qsl, qsl, tmp)
                    base = (h % 2) * 64
                    for st in range(NST):
                        sl = slice(st * STRIP, (st + 1) * STRIP)
                        a_ps = apsp.tile([128, STRIP], f32, tag="aps",
                                         name="a_ps")
                        nc.tensor.matmul(a_ps, ones1_r[base:base + 1, :],
                                         _grow(h, sl), start=True, stop=True)
                        nc.vector.tensor_mul(qTh_view[:, h, sl],
                                             qTh_view[:, h, sl], a_ps)
                # v transposes: 4 per PSUM bank, 4 wide evacuations
                for grp in range(4):
                    vp = vpp.tile([128, 512], f32, tag="vtp", name="vtp")
                    for j in range(4):
                        c = grp * 4 + j
                        nc.tensor.transpose(vp[:, j * 128:(j + 1) * 128],
                                            vT[:, c * 128:(c + 1) * 128], id_f)
                    dst = v_bf[:, grp * 512:(grp + 1) * 512]
                    if grp % 2 == 0:
                        nc.scalar.copy(dst, vp)
                    else:
                        nc.vector.tensor_copy(dst, vp)
                # dense copies of the strided global k/v slices
                vgs = rpool.tile([128, NG], f32, tag="vgs", name="vgs")
                nc.scalar.copy(vgs, vT[:, 0:S:STRIDE])
                nc.scalar.copy(kg, kT[:, 0:S:STRIDE])
                vgp = vpp.tile([32, 128], f32, tag="vgtp", name="vgtp", bufs=1)
                nc.tensor.transpose(vgp, vgs, id_f)
                nc.scalar.copy(vg_bf, vgp)

            # ============ phase 2: attention + output projection ============
            with tc.tile_pool(name="att", bufs=4) as apool, \
                 tc.tile_pool(name="atts", bufs=2) as spool, \
                 tc.tile_pool(name="outp", bufs=4) as opool, \
                 tc.tile_pool(name="ps_s", bufs=3, space="PSUM") as pss, \
                 tc.tile_pool(name="ps_l", bufs=1, space="PSUM") as psl, \
                 tc.tile_pool(name="ps_av", bufs=2, space="PSUM") as psav, \
                 tc.tile_pool(name="ps_wo", bufs=2, space="PSUM") as pswo:
                for qt in range(NQT):
                    q0 = qt * 128
                    wstart, w = _win(qt)
                    nch = w // 128
                    qrhs = qTh_view[:, :, q0:q0 + 128]        # [128, GQ, 128]
                    l_ps = psl.tile([64, 512], f32, tag="lps", name="lps")
                    # ---- local chunks: S^T, mask, exp, l, AV ----
                    av_l = psav.tile([128, 512], f32, tag="av", name="av_l")
                    for c in range(nch):
                        kc = wstart // 128 + c
                        ksl = slice(kc * 128, (kc + 1) * 128)
                        sp = pss.tile([128, 512], f32, tag="sps", name="sps")
                        last = (qt == 0) or (c == nch - 1) or (qt >= 2 and c == 0)
                        nc.tensor.matmul(sp, kT[:, ksl], qrhs,
                                         start=True, stop=not last)
                        if qt >= 2 and c == 0:
                            nc.tensor.matmul(sp, id_r, m_lo, start=False,
                                             stop=True)
                        elif c == nch - 1:
                            nc.tensor.matmul(sp, id_r, m_hi, start=False,
                                             stop=True)
                        pT = apool.tile([128, 512], bf16, tag="pT", name="pT")
                        nc.scalar.activation(pT, sp, EXP, scale=SCALE)
                        nc.tensor.matmul(l_ps[0:1, :], w07_bf, pT,
                                         start=(c == 0), stop=(c == nch - 1))
                        nc.tensor.matmul(av_l, v_bf[:, ksl], pT,
                                         start=(c == 0), stop=(c == nch - 1))
                    # ---- global: S^T_g, exp, l_g, AV_g ----
                    spg = pss.tile([32, 512], f32, tag="sps", name="spg")
                    nc.tensor.matmul(spg, kg, qrhs, start=True, stop=True)
                    pTg = apool.tile([32, 512], bf16, tag="pTg", name="pTg")
                    nc.scalar.activation(pTg, spg, EXP, scale=SCALE)
                    nc.tensor.matmul(l_ps[32:33, :], w03_bf[0:32, :], pTg,
                                     start=True, stop=True)
                    av_g = psav.tile([128, 512], f32, tag="av", name="av_g")
                    nc.tensor.matmul(av_g, vg_bf, pTg, start=True, stop=True)
                    # ---- normalization + 0.7/0.3 mix ----
                    r_l = spool.tile([1, 512], f32r, tag="r_l", name="r_l")
                    r_g = spool.tile([1, 512], f32r, tag="r_g", name="r_g")
                    with nc.allow_low_precision("f32r == f32 bits"):
                        nc.vector.reciprocal(r_l, l_ps[0:1, :])
                        nc.vector.reciprocal(r_g, l_ps[32:33, :])
                    rbp_l = pss.tile([128, 512], f32, tag="sps", name="rbp_l")
                    nc.tensor.matmul(rbp_l, ones1_r[0:1, :], r_l,
                                     start=True, stop=True)
                    rbp_g = pss.tile([128, 512], f32, tag="sps", name="rbp_g")
                    nc.tensor.matmul(rbp_g, ones1_r[0:1, :], r_g,
                                     start=True, stop=True)
                    rb_l = spool.tile([128, 512], f32, tag="rb_l", name="rb_l")
                    rb_g = spool.tile([128, 512], f32, tag="rb_g", name="rb_g")
                    nc.scalar.copy(rb_l, rbp_l)
                    nc.vector.tensor_copy(rb_g, rbp_g)
                    t_l = spool.tile([128, 512], f32, tag="t_l", name="t_l")
                    t_g = spool.tile([128, 512], f32, tag="t_g", name="t_g")
                    nc.vector.tensor_mul(t_l, av_l, rb_l)
                    nc.vector.tensor_mul(t_g, av_g, rb_g)
                    at_all = spool.tile([128, 512], bf16, tag="at", name="at",
                                        bufs=3)
                    nc.gpsimd.tensor_add(at_all, t_l, t_g)
                    # ---- output projection partial for this q tile ----
                    for os_ in range(4):
                        osl = slice(os_ * 512, (os_ + 1) * 512)
                        wo_ps = pswo.tile([128, 512], f32, tag="wops",
                                          name="wops")
                        for h in range(GQ):
                            nc.tensor.matmul(wo_ps,
                                             at_all[:, h * 128:(h + 1) * 128],
                                             woT[h][:, osl],
                                             start=(h == 0), stop=(h == GQ - 1))
                        ot = opool.tile([128, 512], bf16, tag="ot", name="ot")
                        if os_ % 2 == 0:
                            nc.scalar.copy(ot, wo_ps)
                        else:
                            nc.vector.tensor_copy(ot, wo_ps)
                        nc.sync.dma_start(
                            out=po_d[qt // 4, (qt % 4) * 128:(qt % 4) * 128 + 128,
                                     osl],
                            in_=ot)

            # ===== ReduceScatter #2: sum head groups, scatter over seq =====
            nc.gpsimd.collective_compute(
                "ReduceScatter", mybir.AluOpType.add, replica_groups=GROUPS,
                ins=[po_d.opt()], outs=[ro_d.opt()])
            nc.sync.dma_start(out=out_d[:, :], in_=ro_d[:, :])

    nc.finalize()
    return nc


_NC_CACHE = {}


def _get_nc():
    if "nc" not in _NC_CACHE:
        _NC_CACHE["nc"] = _build_nc()
    return _NC_CACHE["nc"]


def kernel(x, Wq, Wkv, Wo, Wr, br):
    bf = ml_dtypes.bfloat16
    x = np.asarray(x)
    xb = x.astype(bf)                              # [2, 2048, 2048]
    wqb = np.asarray(Wq).astype(bf)
    wkvb = np.asarray(Wkv).astype(bf)
    wob = np.asarray(Wo).astype(bf)
    Wr = np.asarray(Wr, dtype=np.float32)
    br = np.asarray(br, dtype=np.float32)

    nc = _get_nc()
    # per-group (g) weight slices are shared by the two batches
    gmaps = []
    for g in range(4):
        cols = slice(g * CSL, (g + 1) * CSL)
        gmaps.append({
            "wq": np.ascontiguousarray(wqb[:, cols]).reshape(16, 128, CSL),
            "wkv": np.ascontiguousarray(wkvb[:, cols]).reshape(8, 128, CSL),
            "wo": np.ascontiguousarray(wob[:, cols]).reshape(16, 128, CSL),
            "wrT": np.ascontiguousarray(Wr[:, cols].T.astype(bf)
                                        ).reshape(NCC, 128, NH),
            "br": np.ascontiguousarray(
                br[g * GQ:(g + 1) * GQ].reshape(GQ, 1)),
        })
    in_maps = []
    for c in range(8):
        b, g = divmod(c, 4)
        cols = slice(g * CSL, (g + 1) * CSL)
        m = dict(gmaps[g])
        m["xg"] = np.ascontiguousarray(xb[b][:, cols]).reshape(16, 128, CSL)
        in_maps.append(m)
    res = run_bass_kernel_spmd(nc, in_maps, list(range(8)))
    out = np.empty((B, S, DIM), dtype=np.float32)
    for c in range(8):
        b, g = divmod(c, 4)
        out[b, g * STRIP:(g + 1) * STRIP, :] = \
            res.results[c]["out"].astype(np.float32)
    return out


# revision 19
# speedup vs baseline: 5.5475x; 1.1693x over previous
"""Trainium2 Bass kernel for nn_CausalSelfAttention_49572512530497.

Sparse attention (local 256-window causal + strided-64 global, GQA 16q/4kv,
RoPE, sigmoid head gating) with fused projections, for B=2, S=2048, DIM=2048.

Sharding: 8 cores = 2 batches x 4 contraction/head-group slices, with
on-device collectives to eliminate input duplication and host-side reduction:

 - Core c=(b,g) receives bf16 slices x[b][:, 512g:512g+512], Wq[:, cols],
   Wkv[:, cols], Wo[:, 512g:512g+512], Wr[:, cols].T  (7 MB/core vs 26 MB
   for head-sharding with replicated x).
 - Phase 1 computes PARTIAL q/k/v/gate projections for ALL 16 heads of
   batch b (contraction over its 512-column slice of DIM), laid out in
   DRAM as 4 head-group chunks.
 - ReduceScatter(add) over the 4 cores of each batch delivers to core
   (b,g) the COMPLETE q (4 heads), k/v (kv head g) and gate logits.
 - Phase 2 = RoPE + sigmoid gating + windowed-local+strided-global
   attention + output projection partial (same instruction structure as
   the head-sharded kernel: f32r score matmuls, PE-applied additive
   masks, shared-PSUM softmax, bf16 AV, diag(1/l) normalization).
 - A second ReduceScatter over sequence chunks gives each core a disjoint
   [512, 2048] slice of the final output: D2H is 32 MB total, no host sum.

All input tensors ship as bf16 (PE products of bf16 operands accumulate
exactly in f32, so matmul precision matches f32 compute on bf16-quantized
data); device-side PE transposes produce the [contraction, free] layouts,
so the host does no large transposes.
"""

import numpy as np
import ml_dtypes

import concourse.bass as bass
import concourse.mybir as mybir
import concourse.tile as tile
from concourse import bacc
from concourse.bass_utils import run_bass_kernel_spmd

B, S, DIM = 2, 2048, 2048
NH, NKV = 16, 4
HD = DIM // NH            # 128
GQ = NH // NKV            # 4 q-heads per kv head / per core
BASE = 10000.0
WINDOW, STRIDE = 256, 64
NG = S // STRIDE          # 32 global keys
SCALE = 1.0 / float(np.sqrt(HD))
NQT = S // 128            # 16 query tiles
CSL = DIM // 4            # 512 contraction columns per core
NCC = CSL // 128          # 4 contraction chunks
NST = 4                   # seq strips for projections
STRIP = S // NST          # 512
MASKVAL = -1e30
GROUPS = [[0, 1, 2, 3], [4, 5, 6, 7]]
# partial chunk layout (rows): q heads 4g..4g+4 | k head g | v head g | gates
PQ, PK, PV, PGT = 0, 512, 640, 768
PROWS = 772

f32 = mybir.dt.float32
f32r = mybir.dt.float32r
bf16 = mybir.dt.bfloat16
EXP = mybir.ActivationFunctionType.Exp
SIGMOID = mybir.ActivationFunctionType.Sigmoid


def _rope_tables():
    half = HD // 2
    inv_freq = 1.0 / (BASE ** (np.arange(0, half, dtype=np.float64) * 2.0 / HD))
    t = np.arange(S, dtype=np.float64)
    freqs = t[:, None] * inv_freq[None, :]          # [S, 64]
    cosT = np.cos(freqs).T.astype(np.float32)       # [64, S]
    sinT = np.sin(freqs).T.astype(np.float32)
    cos2 = np.concatenate([cosT, cosT], axis=0)     # [128, S]
    sin2s = np.concatenate([-sinT, sinT], axis=0)   # [128, S]
    return cos2, sin2s


def _win(qt):
    q0 = qt * 128
    wstart = max(0, q0 - WINDOW)
    return wstart, q0 + 128 - wstart


def _build_nc():
    nc = bacc.Bacc(num_devices=8)

    xg_d = nc.dram_tensor("xg", [16, 128, CSL], bf16, kind="ExternalInput")
    # wh = this core's HALF of the weight slices (batch 0 cores ship the
    # first 1024 rows, batch 1 cores the rest); a pairwise AllGather
    # between cores (g, 4+g) reconstructs the full slices on device.
    # blocks 0..7 = Wq rows, 8..11 = Wkv rows, 12..19 = Wo rows.
    wh_d = nc.dram_tensor("wh", [20, 128, CSL], bf16, kind="ExternalInput")
    wrT_d = nc.dram_tensor("wrT", [NCC, 128, NH], bf16, kind="ExternalInput")
    br_d = nc.dram_tensor("br", [GQ, 1], f32, kind="ExternalInput")
    out_d = nc.dram_tensor("out", [STRIP, DIM], bf16, kind="ExternalOutput")

    cos2_np, sin2s_np = _rope_tables()
    cos2_d = nc.inline_tensor(cos2_np, "cos2c")
    sin2s_d = nc.inline_tensor(sin2s_np, "sin2sc")
    kj = np.arange(128)[:, None]
    qi = np.arange(128)[None, :]
    mlo = np.where(kj >= qi, 0.0, MASKVAL).astype(np.float32)   # first window chunk
    mhi = np.where(kj <= qi, 0.0, MASKVAL).astype(np.float32)   # diagonal chunk
    mlo_d = nc.inline_tensor(np.tile(mlo, (1, GQ)), "mloc")     # [128, 512]
    mhi_d = nc.inline_tensor(np.tile(mhi, (1, GQ)), "mhic")
    eye = np.eye(128)
    idf_d = nc.inline_tensor(eye.astype(np.float32), "idfc")
    idb_d = nc.inline_tensor(eye.astype(ml_dtypes.bfloat16), "idbc")
    w07_d = nc.inline_tensor(np.full((128, 1), 1.0 / 0.7, ml_dtypes.bfloat16),
                             "w07c")
    w03_d = nc.inline_tensor(np.full((128, 1), 1.0 / 0.3, ml_dtypes.bfloat16),
                             "w03c")
    ones1_d = nc.inline_tensor(np.ones((128, 128), np.float32), "ones1c")

    with tile.TileContext(nc) as tc:
        with tc.tile_pool(name="glob", bufs=1) as glob, \
             tc.tile_pool(name="gdram", bufs=1, space="DRAM") as gdram:
            part_d = gdram.tile([GQ, PROWS, S], bf16, name="part_d")
            rs_d = gdram.tile([PROWS, S], bf16, name="rs_d")
            whb_d = gdram.tile([20, 128, CSL], bf16, name="whb_d")
            wg_d = gdram.tile([40, 128, CSL], bf16, name="wg_d")
            po_d = gdram.tile([GQ, STRIP, DIM], bf16, name="po_d")
            ro_d = gdram.tile([STRIP, DIM], bf16, name="ro_d")

            qT = glob.tile([128, GQ * S], f32r, tag="qTa", name="qTa")
            qTh_view = qT.rearrange("p (h s) -> p h s", h=GQ)
            kT = glob.tile([128, S], f32r, tag="kT", name="kT")
            vT = glob.tile([128, S], f32, tag="vT", name="vT")
            v_bf = glob.tile([128, S], bf16, tag="v_bf", name="v_bf")
            vg_bf = glob.tile([32, 128], bf16, tag="vgbf", name="vgbf")
            kg = glob.tile([128, NG], f32r, tag="kg", name="kg")
            gateS = glob.tile([GQ, S], f32r, tag="gateS", name="gateS")
            gAB = [glob.tile([65, S], f32r, tag=f"gAB{i}", name=f"gAB{i}")
                   for i in range(2)]
            def _grow(h, sl=slice(None)):
                return gAB[h // 2][(h % 2) * 64:(h % 2) * 64 + 1, sl]
            cos2 = glob.tile([128, S], f32, tag="cos2", name="cos2")
            sin2s = glob.tile([128, S], f32, tag="sin2s", name="sin2s")
            m_lo = glob.tile([128, 512], f32r, tag="m_lo", name="m_lo")
            m_hi = glob.tile([128, 512], f32r, tag="m_hi", name="m_hi")
            id_f = glob.tile([128, 128], f32, tag="idf", name="idf")
            id_r = glob.tile([128, 128], f32r, tag="idr", name="idr")
            id_b = glob.tile([128, 128], bf16, tag="idb", name="idb")
            w07_bf = glob.tile([128, 1], bf16, tag="w07bf", name="w07bf")
            w03_bf = glob.tile([128, 1], bf16, tag="w03bf", name="w03bf")
            ones1_r = glob.tile([128, 128], f32r, tag="ones1r", name="ones1r")
            br_t = glob.tile([GQ, 1], f32, tag="br", name="br")
            woT = [glob.tile([128, DIM], bf16, tag=f"wo{h}", name=f"wo{h}")
                   for h in range(GQ)]

            nc.sync.dma_start(out=br_t, in_=br_d[:, :])
            nc.sync.dma_start(out=cos2, in_=cos2_d[:, :])
            nc.sync.dma_start(out=sin2s, in_=sin2s_d[:, :])
            nc.sync.dma_start(out=ones1_r, in_=ones1_d[:, :].bitcast(f32r))
            nc.sync.dma_start(out=id_f, in_=idf_d[:, :])
            nc.sync.dma_start(out=id_r, in_=idf_d[:, :].bitcast(f32r))
            nc.sync.dma_start(out=id_b, in_=idb_d[:, :])
            nc.sync.dma_start(out=m_lo, in_=mlo_d[:, :].bitcast(f32r))
            nc.sync.dma_start(out=m_hi, in_=mhi_d[:, :].bitcast(f32r))
            nc.sync.dma_start(out=w07_bf, in_=w07_d[:, :])
            nc.sync.dma_start(out=w03_bf, in_=w03_d[:, :])

            # ===== phase 0: stage bf16 inputs, PE-transpose to [contr, free] =====
            # xT[cc]   [128, S]    : x[b][:, cols].T chunk cc
            # wqT[cc]  [128, 2048] : Wq[:, cols].T   (free = q out-dims, 16 heads)
            # wkvT[cc] [128, 1024] : Wkv[:, cols].T  (free = k heads | v heads)
            # woT[h]   [128, 2048] : Wo[:, 512g+128h ...].T (free = out-dims)
            with tc.tile_pool(name="ph1", bufs=1) as ph1:
                xT = [ph1.tile([128, S], bf16, tag=f"xT{c}", name=f"xT{c}")
                      for c in range(NCC)]
                wqT = [ph1.tile([128, DIM], bf16, tag=f"wqT{c}", name=f"wqT{c}")
                       for c in range(NCC)]
                wkvT = [ph1.tile([128, 1024], bf16, tag=f"wkvT{c}",
                                 name=f"wkvT{c}") for c in range(NCC)]
                wr_sb = [ph1.tile([128, NH], bf16, tag=f"wr{c}", name=f"wr{c}")
                         for c in range(NCC)]
                for c in range(NCC):
                    nc.sync.dma_start(out=wr_sb[c], in_=wrT_d[c])

                # pairwise AllGather reconstructs full weight slices
                nc.sync.dma_start(out=whb_d[:, :, :], in_=wh_d[:, :, :])
                nc.gpsimd.collective_compute(
                    "AllGather", mybir.AluOpType.bypass,
                    replica_groups=[[0, 4], [1, 5], [2, 6], [3, 7]],
                    ins=[whb_d.opt()], outs=[wg_d.opt()])
                wq_blk = list(range(0, 8)) + list(range(20, 28))
                wkv_blk = list(range(8, 12)) + list(range(28, 32))
                wo_blk = list(range(12, 20)) + list(range(32, 40))

                with tc.tile_pool(name="stage", bufs=2) as stage, \
                     tc.tile_pool(name="tps", bufs=4, space="PSUM") as tps:
                    eng = [nc.scalar.copy, nc.vector.tensor_copy]
                    ei = 0
                    for blocks, dsts in (
                            ([xg_d[i] for i in range(16)], xT),
                            ([wg_d[i] for i in wq_blk], wqT),
                            ([wg_d[i] for i in wkv_blk], wkvT),
                            ([wg_d[i] for i in wo_blk], woT)):
                        nb = len(blocks)
                        st_t = stage.tile([128, 16 * CSL], bf16, tag="stg",
                                          name="stg")
                        for sb in range(nb):
                            nc.sync.dma_start(
                                out=st_t[:, sb * CSL:(sb + 1) * CSL],
                                in_=blocks[sb])
                        for cc in range(NCC):
                            if dsts is woT:
                                dst = woT[cc]          # head h = hd-chunk cc
                            else:
                                dst = dsts[cc]
                            for grp in range(nb // 4):
                                pt = tps.tile([128, 512], bf16, tag="tp",
                                              name="tp")
                                for j in range(4):
                                    sb = grp * 4 + j
                                    nc.tensor.transpose(
                                        pt[:, j * 128:(j + 1) * 128],
                                        st_t[:, sb * CSL + cc * 128:
                                             sb * CSL + cc * 128 + 128],
                                        id_b)
                                eng[ei % 2](dst[:, grp * 512:(grp + 1) * 512],
                                            pt)
                                ei += 1

                # ===== phase 1: partial projections for all 16 heads =====
                with tc.tile_pool(name="pps", bufs=4, space="PSUM") as ppool, \
                     tc.tile_pool(name="gps", bufs=2, space="PSUM") as gpool, \
                     tc.tile_pool(name="pev", bufs=6) as epool:
                    ei = 0
                    for st in range(NST):
                        sl = slice(st * STRIP, (st + 1) * STRIP)
                        # q blocks (head h) and kv blocks
                        for ob in range(24):
                            pp = ppool.tile([128, STRIP], f32, tag="pp",
                                            name="pp")
                            for cc in range(NCC):
                                if ob < 16:
                                    stat = wqT[cc][:, ob * 128:(ob + 1) * 128]
                                else:
                                    kb = ob - 16
                                    stat = wkvT[cc][:, kb * 128:(kb + 1) * 128]
                                nc.tensor.matmul(pp, stat, xT[cc][:, sl],
                                                 start=(cc == 0),
                                                 stop=(cc == NCC - 1))
                            ev = epool.tile([128, STRIP], bf16, tag="ev",
                                            name="ev")
                            eng[ei % 2](ev, pp)
                            ei += 1
                            if ob < 16:          # q head ob
                                gg, r0 = ob // GQ, PQ + (ob % GQ) * 128
                            elif ob < 20:        # k head ob-16
                                gg, r0 = ob - 16, PK
                            else:                # v head ob-20
                                gg, r0 = ob - 20, PV
                            nc.sync.dma_start(
                                out=part_d[gg, r0:r0 + 128, sl], in_=ev)
                        # gate logits for all 16 heads
                        gp = gpool.tile([NH, STRIP], f32, tag="gp", name="gp")
                        for cc in range(NCC):
                            nc.tensor.matmul(gp, wr_sb[cc], xT[cc][:, sl],
                                             start=(cc == 0),
                                             stop=(cc == NCC - 1))
                        gev = epool.tile([NH, STRIP], bf16, tag="gev",
                                         name="gev")
                        nc.vector.tensor_copy(gev, gp)
                        for gg in range(GQ):
                            nc.sync.dma_start(
                                out=part_d[gg, PGT:PGT + GQ, sl],
                                in_=gev[gg * GQ:(gg + 1) * GQ, :])

            # ===== ReduceScatter #1: complete q/k/v/gate for own head group ====
            nc.gpsimd.collective_compute(
                "ReduceScatter", mybir.AluOpType.add, replica_groups=GROUPS,
                ins=[part_d.opt()], outs=[rs_d.opt()])

            # ===== phase 1.5: sigmoid gate, RoPE, v transposes, global k/v ====
            with tc.tile_pool(name="rtmp", bufs=3) as rpool, \
                 tc.tile_pool(name="rsst", bufs=1) as rsst, \
                 tc.tile_pool(name="aps", bufs=2, space="PSUM") as apsp, \
                 tc.tile_pool(name="vtps", bufs=2, space="PSUM") as vpp:
                # stage the bf16 RS output, upconvert to f32r working tiles
                qs_bf = rsst.tile([128, GQ * S], bf16, tag="qs_bf",
                                  name="qs_bf")
                kv_bf = rsst.tile([128, 2 * S], bf16, tag="kv_bf",
                                  name="kv_bf")
                g_bfs = rsst.tile([GQ, S], bf16, tag="g_bfs", name="g_bfs")
                glogit = rsst.tile([GQ, S], f32, tag="glogit", name="glogit")
                for h in range(GQ):
                    nc.sync.dma_start(
                        out=qs_bf[:, h * S:(h + 1) * S],
                        in_=rs_d[PQ + h * 128:PQ + (h + 1) * 128, :])
                nc.sync.dma_start(out=kv_bf[:, 0:S], in_=rs_d[PK:PK + 128, :])
                nc.sync.dma_start(out=kv_bf[:, S:2 * S],
                                  in_=rs_d[PV:PV + 128, :])
                nc.sync.dma_start(out=g_bfs, in_=rs_d[PGT:PGT + GQ, :])
                for h in range(GQ):
                    eng[h % 2](qTh_view[:, h, :], qs_bf[:, h * S:(h + 1) * S])
                nc.scalar.copy(kT, kv_bf[:, 0:S])
                nc.vector.tensor_copy(vT, kv_bf[:, S:2 * S])
                nc.scalar.copy(glogit, g_bfs)
                nc.scalar.activation(gateS, glogit, SIGMOID, bias=br_t,
                                     scale=1.0)
                for h in range(GQ):
                    nc.sync.dma_start(out=_grow(h), in_=gateS[h:h + 1, :])
                # RoPE k (in place): k = k*cos + swap(k)*[-sin; sin].
                # The swapped-half copy comes from a partition-shifting
                # SBUF DMA so every DVE operand pair is partition-aligned.
                swp = rpool.tile([128, S], f32, tag="swp", name="swp")
                nc.sync.dma_start(out=swp[0:64], in_=kT[64:128].bitcast(f32))
                nc.sync.dma_start(out=swp[64:128], in_=kT[0:64].bitcast(f32))
                tmp = rpool.tile([128, S], f32, tag="ropetmp", name="ropetmp")
                nc.vector.tensor_mul(tmp, swp, sin2s)
                nc.vector.tensor_mul(kT, kT, cos2)
                nc.gpsimd.tensor_add(kT, kT, tmp)
                # RoPE q + sigmoid gate fold (PE-broadcast gate rows)
                for h in range(GQ):
                    qsl = qTh_view[:, h, :]
                    swp = rpool.tile([128, S], f32, tag="swp", name="swp")
                    nc.sync.dma_start(out=swp[0:64],
                                      in_=qsl[64:128].bitcast(f32))
                    nc.sync.dma_start(out=swp[64:128],
                                      in_=qsl[0:64].bitcast(f32))
                    tmp = rpool.tile([128, S], f32, tag="ropetmp",
                                     name="ropetmp")
                    nc.vector.tensor_mul(tmp, swp, sin2s)
                    nc.vector.tensor_mul(qsl, qsl, cos2)
                    nc.gpsimd.tensor_add(qsl, qsl, tmp)
                    base = (h % 2) * 64
                    for st in range(NST):
                        sl = slice(st * STRIP, (st + 1) * STRIP)
                        a_ps = apsp.tile([128, STRIP], f32, tag="aps",
                                         name="a_ps")
                        nc.tensor.matmul(a_ps, ones1_r[base:base + 1, :],
                                         _grow(h, sl), start=True, stop=True)
                        nc.vector.tensor_mul(qTh_view[:, h, sl],
                                             qTh_view[:, h, sl], a_ps)
                # v transposes: 4 per PSUM bank, 4 wide evacuations
                for grp in range(4):
                    vp = vpp.tile([128, 512], f32, tag="vtp", name="vtp")
                    for j in range(4):
                        c = grp * 4 + j
                        nc.tensor.transpose(vp[:, j * 128:(j + 1) * 128],
                                            vT[:, c * 128:(c + 1) * 128], id_f)
                    dst = v_bf[:, grp * 512:(grp + 1) * 512]
                    if grp % 2 == 0:
                        nc.scalar.copy(dst, vp)
                    else:
                        nc.vector.tensor_copy(dst, vp)
                # dense copies of the strided global k/v slices
                vgs = rpool.tile([128, NG], f32, tag="vgs", name="vgs")
                nc.scalar.copy(vgs, vT[:, 0:S:STRIDE])
                nc.scalar.copy(kg, kT[:, 0:S:STRIDE])
                vgp = vpp.tile([32, 128], f32, tag="vgtp", name="vgtp", bufs=1)
                nc.tensor.transpose(vgp, vgs, id_f)
                nc.scalar.copy(vg_bf, vgp)

            # ============ phase 2: attention + output projection ============
            with tc.tile_pool(name="att", bufs=4) as apool, \
                 tc.tile_pool(name="atts", bufs=2) as spool, \
                 tc.tile_pool(name="outp", bufs=4) as opool, \
                 tc.tile_pool(name="ps_s", bufs=3, space="PSUM") as pss, \
                 tc.tile_pool(name="ps_l", bufs=1, space="PSUM") as psl, \
                 tc.tile_pool(name="ps_av", bufs=2, space="PSUM") as psav, \
                 tc.tile_pool(name="ps_wo", bufs=2, space="PSUM") as pswo:
                for qt in range(NQT):
                    q0 = qt * 128
                    wstart, w = _win(qt)
                    nch = w // 128
                    qrhs = qTh_view[:, :, q0:q0 + 128]        # [128, GQ, 128]
                    l_ps = psl.tile([64, 512], f32, tag="lps", name="lps")
                    # ---- local chunks: S^T, mask, exp, l, AV ----
                    av_l = psav.tile([128, 512], f32, tag="av", name="av_l")
                    for c in range(nch):
                        kc = wstart // 128 + c
                        ksl = slice(kc * 128, (kc + 1) * 128)
                        sp = pss.tile([128, 512], f32, tag="sps", name="sps")
                        last = (qt == 0) or (c == nch - 1) or (qt >= 2 and c == 0)
                        nc.tensor.matmul(sp, kT[:, ksl], qrhs,
                                         start=True, stop=not last)
                        if qt >= 2 and c == 0:
                            nc.tensor.matmul(sp, id_r, m_lo, start=False,
                                             stop=True)
                        elif c == nch - 1:
                            nc.tensor.matmul(sp, id_r, m_hi, start=False,
                                             stop=True)
                        pT = apool.tile([128, 512], bf16, tag="pT", name="pT")
                        nc.scalar.activation(pT, sp, EXP, scale=SCALE)
                        nc.tensor.matmul(l_ps[0:1, :], w07_bf, pT,
                                         start=(c == 0), stop=(c == nch - 1))
                        nc.tensor.matmul(av_l, v_bf[:, ksl], pT,
                                         start=(c == 0), stop=(c == nch - 1))
                    # ---- global: S^T_g, exp, l_g, AV_g ----
                    spg = pss.tile([32, 512], f32, tag="sps", name="spg")
                    nc.tensor.matmul(spg, kg, qrhs, start=True, stop=True)
                    pTg = apool.tile([32, 512], bf16, tag="pTg", name="pTg")
                    nc.scalar.activation(pTg, spg, EXP, scale=SCALE)
                    nc.tensor.matmul(l_ps[32:33, :], w03_bf[0:32, :], pTg,
                                     start=True, stop=True)
                    av_g = psav.tile([128, 512], f32, tag="av", name="av_g")
                    nc.tensor.matmul(av_g, vg_bf, pTg, start=True, stop=True)
                    # ---- normalization + 0.7/0.3 mix ----
                    r_l = spool.tile([1, 512], f32r, tag="r_l", name="r_l")
                    r_g = spool.tile([1, 512], f32r, tag="r_g", name="r_g")
                    with nc.allow_low_precision("f32r == f32 bits"):
                        nc.vector.reciprocal(r_l, l_ps[0:1, :])
                        nc.vector.reciprocal(r_g, l_ps[32:33, :])
                    rbp_l = pss.tile([128, 512], f32, tag="sps", name="rbp_l")
                    nc.tensor.matmul(rbp_l, ones1_r[0:1, :], r_l,
                                     start=True, stop=True)
                    rbp_g = pss.tile([128, 512], f32, tag="sps", name="rbp_g")
                    nc.tensor.matmul(rbp_g, ones1_r[0:1, :], r_g,
                                     start=True, stop=True)
                    rb_l = spool.tile([128, 512], f32, tag="rb_l", name="rb_l")
                    rb_g = spool.tile([128, 512], f32, tag="rb_g", name="rb_g")
                    nc.scalar.copy(rb_l, rbp_l)
                    nc.vector.tensor_copy(rb_g, rbp_g)
                    t_l = spool.tile([128, 512], f32, tag="t_l", name="t_l")
                    t_g = spool.tile([128, 512], f32, tag="t_g", name="t_g")
                    nc.vector.tensor_mul(t_l, av_l, rb_l)
                    nc.vector.tensor_mul(t_g, av_g, rb_g)
                    at_all = spool.tile([128, 512], bf16, tag="at", name="at",
                                        bufs=3)
                    nc.gpsimd.tensor_add(at_all, t_l, t_g)
                    # ---- output projection partial for this q tile ----
                    for os_ in range(4):
                        osl = slice(os_ * 512, (os_ + 1) * 512)
                        wo_ps = pswo.tile([128, 512], f32, tag="wops",
                                          name="wops")
                        for h in range(GQ):
                            nc.tensor.matmul(wo_ps,
                                             at_all[:, h * 128:(h + 1) * 128],
                                             woT[h][:, osl],
                                             start=(h == 0), stop=(h == GQ - 1))
                        ot = opool.tile([128, 512], bf16, tag="ot", name="ot")
                        if os_ % 2 == 0:
                            nc.scalar.copy(ot, wo_ps)
                        else:
                            nc.vector.tensor_copy(ot, wo_ps)
                        nc.sync.dma_start(
                            out=po_d[qt // 4, (qt % 4) * 128:(qt % 4) * 128 + 128,
                                     osl],
                            in_=ot)

            # ===== ReduceScatter #2: sum head groups, scatter over seq =====
            nc.gpsimd.collective_compute(
                "ReduceScatter", mybir.AluOpType.add, replica_groups=GROUPS,
                ins=[po_d.opt()], outs=[ro_d.opt()])
            nc.sync.dma_start(out=out_d[:, :], in_=ro_d[:, :])

    nc.finalize()
    return nc


_NC_CACHE = {}


def _get_nc():
    if "nc" not in _NC_CACHE:
        _NC_CACHE["nc"] = _build_nc()
    return _NC_CACHE["nc"]


def kernel(x, Wq, Wkv, Wo, Wr, br):
    bf = ml_dtypes.bfloat16
    x = np.asarray(x)
    xb = x.astype(bf)                              # [2, 2048, 2048]
    wqb = np.asarray(Wq).astype(bf)
    wkvb = np.asarray(Wkv).astype(bf)
    wob = np.asarray(Wo).astype(bf)
    Wr = np.asarray(Wr, dtype=np.float32)
    br = np.asarray(br, dtype=np.float32)

    nc = _get_nc()
    in_maps = []
    for c in range(8):
        b, g = divmod(c, 4)
        cols = slice(g * CSL, (g + 1) * CSL)
        # this core's half of the weight slices (other half comes from the
        # paired core via an on-device AllGather)
        wh = np.empty((20, 128, CSL), dtype=bf)
        wh[0:8] = wqb[b * 1024:(b + 1) * 1024, cols].reshape(8, 128, CSL)
        wh[8:12] = wkvb[b * 512:(b + 1) * 512, cols].reshape(4, 128, CSL)
        wh[12:20] = wob[b * 1024:(b + 1) * 1024, cols].reshape(8, 128, CSL)
        in_maps.append({
            "xg": np.ascontiguousarray(xb[b][:, cols]).reshape(16, 128, CSL),
            "wh": wh,
            "wrT": np.ascontiguousarray(Wr[:, cols].T.astype(bf)
                                        ).reshape(NCC, 128, NH),
            "br": np.ascontiguousarray(
                br[g * GQ:(g + 1) * GQ].reshape(GQ, 1)),
        })
    res = run_bass_kernel_spmd(nc, in_maps, list(range(8)))
    out = np.empty((B, S, DIM), dtype=np.float32)
    for c in range(8):
        b, g = divmod(c, 4)
        out[b, g * STRIP:(g + 1) * STRIP, :] = \
            res.results[c]["out"].astype(np.float32)
    return out


# revision 20
# speedup vs baseline: 6.2134x; 1.1200x over previous
"""Trainium2 Bass kernel for nn_CausalSelfAttention_49572512530497.

Sparse attention (local 256-window causal + strided-64 global, GQA 16q/4kv,
RoPE, sigmoid head gating) with fused projections, for B=2, S=2048, DIM=2048.

Sharding: 8 cores = 2 batches x 4 contraction/head-group slices, with
on-device collectives to eliminate input duplication and host-side reduction:

 - Core c=(b,g) receives bf16 slices x[b][:, 512g:512g+512], Wq[:, cols],
   Wkv[:, cols], Wo[:, 512g:512g+512], Wr[:, cols].T  (7 MB/core vs 26 MB
   for head-sharding with replicated x).
 - Phase 1 computes PARTIAL q/k/v/gate projections for ALL 16 heads of
   batch b (contraction over its 512-column slice of DIM), laid out in
   DRAM as 4 head-group chunks.
 - ReduceScatter(add) over the 4 cores of each batch delivers to core
   (b,g) the COMPLETE q (4 heads), k/v (kv head g) and gate logits.
 - Phase 2 = RoPE + sigmoid gating + windowed-local+strided-global
   attention + output projection partial (same instruction structure as
   the head-sharded kernel: f32r score matmuls, PE-applied additive
   masks, shared-PSUM softmax, bf16 AV, diag(1/l) normalization).
 - A second ReduceScatter over sequence chunks gives each core a disjoint
   [512, 2048] slice of the final output: D2H is 32 MB total, no host sum.

All input tensors ship as bf16 (PE products of bf16 operands accumulate
exactly in f32, so matmul precision matches f32 compute on bf16-quantized
data); device-side PE transposes produce the [contraction, free] layouts,
so the host does no large transposes.
"""

import numpy as np
import ml_dtypes

import concourse.bass as bass
import concourse.mybir as mybir
import concourse.tile as tile
from concourse import bacc
from concourse.bass_utils import run_bass_kernel_spmd

B, S, DIM = 2, 2048, 2048
NH, NKV = 16, 4
HD = DIM // NH            # 128
GQ = NH // NKV            # 4 q-heads per kv head / per core
BASE = 10000.0
WINDOW, STRIDE = 256, 64
NG = S // STRIDE          # 32 global keys
SCALE = 1.0 / float(np.sqrt(HD))
NQT = S // 128            # 16 query tiles
CSL = DIM // 4            # 512 contraction columns per core
NCC = CSL // 128          # 4 contraction chunks
NST = 4                   # seq strips for projections
STRIP = S // NST          # 512
MASKVAL = -1e30
GROUPS = [[0, 1, 2, 3], [4, 5, 6, 7]]
# partial chunk layout (rows): q heads 4g..4g+4 | k head g | v head g | gates
PQ, PK, PV, PGT = 0, 512, 640, 768
PROWS = 772

f32 = mybir.dt.float32
f32r = mybir.dt.float32r
bf16 = mybir.dt.bfloat16
EXP = mybir.ActivationFunctionType.Exp
SIGMOID = mybir.ActivationFunctionType.Sigmoid


def _rope_tables():
    half = HD // 2
    inv_freq = 1.0 / (BASE ** (np.arange(0, half, dtype=np.float64) * 2.0 / HD))
    t = np.arange(S, dtype=np.float64)
    freqs = t[:, None] * inv_freq[None, :]          # [S, 64]
    cosT = np.cos(freqs).T.astype(np.float32)       # [64, S]
    sinT = np.sin(freqs).T.astype(np.float32)
    cos2 = np.concatenate([cosT, cosT], axis=0)     # [128, S]
    sin2s = np.concatenate([-sinT, sinT], axis=0)   # [128, S]
    return cos2, sin2s


def _win(qt):
    q0 = qt * 128
    wstart = max(0, q0 - WINDOW)
    return wstart, q0 + 128 - wstart


def _build_nc():
    nc = bacc.Bacc(num_devices=8)

    xg_d = nc.dram_tensor("xg", [16, 128, CSL], bf16, kind="ExternalInput")
    # wh = this core's HALF of the weight slices (batch 0 cores ship the
    # first 1024 rows, batch 1 cores the rest); a pairwise AllGather
    # between cores (g, 4+g) reconstructs the full slices on device.
    # blocks 0..7 = Wq rows, 8..11 = Wkv rows, 12..19 = Wo rows.
    wh_d = nc.dram_tensor("wh", [20, 128, CSL], bf16, kind="ExternalInput")
    wrT_d = nc.dram_tensor("wrT", [NCC, 128, NH], bf16, kind="ExternalInput")
    br_d = nc.dram_tensor("br", [GQ, 1], f32, kind="ExternalInput")
    out_d = nc.dram_tensor("out", [STRIP, DIM], bf16, kind="ExternalOutput")

    cos2_np, sin2s_np = _rope_tables()
    cos2_d = nc.inline_tensor(cos2_np, "cos2c")
    sin2s_d = nc.inline_tensor(sin2s_np, "sin2sc")
    kj = np.arange(128)[:, None]
    qi = np.arange(128)[None, :]
    mlo = np.where(kj >= qi, 0.0, MASKVAL).astype(np.float32)   # first window chunk
    mhi = np.where(kj <= qi, 0.0, MASKVAL).astype(np.float32)   # diagonal chunk
    mlo_d = nc.inline_tensor(np.tile(mlo, (1, GQ)), "mloc")     # [128, 512]
    mhi_d = nc.inline_tensor(np.tile(mhi, (1, GQ)), "mhic")
    eye = np.eye(128)
    idf_d = nc.inline_tensor(eye.astype(np.float32), "idfc")
    idb_d = nc.inline_tensor(eye.astype(ml_dtypes.bfloat16), "idbc")
    w07_d = nc.inline_tensor(np.full((128, 1), 1.0 / 0.7, ml_dtypes.bfloat16),
                             "w07c")
    w03_d = nc.inline_tensor(np.full((128, 1), 1.0 / 0.3, ml_dtypes.bfloat16),
                             "w03c")
    ones1_d = nc.inline_tensor(np.ones((128, 128), np.float32), "ones1c")

    with tile.TileContext(nc) as tc:
        with tc.tile_pool(name="glob", bufs=1) as glob, \
             tc.tile_pool(name="gdram", bufs=1, space="DRAM") as gdram:
            part_d = gdram.tile([GQ, PROWS, S], bf16, name="part_d")
            rs_d = gdram.tile([PROWS, S], bf16, name="rs_d")
            whb_d = gdram.tile([20, 128, CSL], bf16, name="whb_d")
            wg_d = gdram.tile([40, 128, CSL], bf16, name="wg_d")
            po_d = gdram.tile([GQ, STRIP, DIM], bf16, name="po_d")
            ro_d = gdram.tile([STRIP, DIM], bf16, name="ro_d")

            qT = glob.tile([128, GQ * S], f32r, tag="qTa", name="qTa")
            qTh_view = qT.rearrange("p (h s) -> p h s", h=GQ)
            kT = glob.tile([128, S], f32r, tag="kT", name="kT")
            vT = glob.tile([128, S], f32, tag="vT", name="vT")
            v_bf = glob.tile([128, S], bf16, tag="v_bf", name="v_bf")
            vg_bf = glob.tile([32, 128], bf16, tag="vgbf", name="vgbf")
            kg = glob.tile([128, NG], f32r, tag="kg", name="kg")
            gateS = glob.tile([GQ, S], f32r, tag="gateS", name="gateS")
            gAB = [glob.tile([65, S], f32r, tag=f"gAB{i}", name=f"gAB{i}")
                   for i in range(2)]
            def _grow(h, sl=slice(None)):
                return gAB[h // 2][(h % 2) * 64:(h % 2) * 64 + 1, sl]
            cos2 = glob.tile([128, S], f32, tag="cos2", name="cos2")
            sin2s = glob.tile([128, S], f32, tag="sin2s", name="sin2s")
            m_lo = glob.tile([128, 512], f32r, tag="m_lo", name="m_lo")
            m_hi = glob.tile([128, 512], f32r, tag="m_hi", name="m_hi")
            id_f = glob.tile([128, 128], f32, tag="idf", name="idf")
            id_r = glob.tile([128, 128], f32r, tag="idr", name="idr")
            id_b = glob.tile([128, 128], bf16, tag="idb", name="idb")
            w07_bf = glob.tile([128, 1], bf16, tag="w07bf", name="w07bf")
            w03_bf = glob.tile([128, 1], bf16, tag="w03bf", name="w03bf")
            ones1_r = glob.tile([128, 128], f32r, tag="ones1r", name="ones1r")
            br_t = glob.tile([GQ, 1], f32, tag="br", name="br")
            woT = [glob.tile([128, DIM], bf16, tag=f"wo{h}", name=f"wo{h}")
                   for h in range(GQ)]

            nc.sync.dma_start(out=br_t, in_=br_d[:, :])
            nc.sync.dma_start(out=cos2, in_=cos2_d[:, :])
            nc.sync.dma_start(out=sin2s, in_=sin2s_d[:, :])
            nc.sync.dma_start(out=ones1_r, in_=ones1_d[:, :].bitcast(f32r))
            nc.sync.dma_start(out=id_f, in_=idf_d[:, :])
            nc.sync.dma_start(out=id_r, in_=idf_d[:, :].bitcast(f32r))
            nc.sync.dma_start(out=id_b, in_=idb_d[:, :])
            nc.sync.dma_start(out=m_lo, in_=mlo_d[:, :].bitcast(f32r))
            nc.sync.dma_start(out=m_hi, in_=mhi_d[:, :].bitcast(f32r))
            nc.sync.dma_start(out=w07_bf, in_=w07_d[:, :])
            nc.sync.dma_start(out=w03_bf, in_=w03_d[:, :])

            # ===== phase 0: stage bf16 inputs, PE-transpose to [contr, free] =====
            # xT[cc]   [128, S]    : x[b][:, cols].T chunk cc
            # wqT[cc]  [128, 2048] : Wq[:, cols].T   (free = q out-dims, 16 heads)
            # wkvT[cc] [128, 1024] : Wkv[:, cols].T  (free = k heads | v heads)
            # woT[h]   [128, 2048] : Wo[:, 512g+128h ...].T (free = out-dims)
            with tc.tile_pool(name="ph1", bufs=1) as ph1:
                xT = [ph1.tile([128, S], bf16, tag=f"xT{c}", name=f"xT{c}")
                      for c in range(NCC)]
                wqT = [ph1.tile([128, DIM], bf16, tag=f"wqT{c}", name=f"wqT{c}")
                       for c in range(NCC)]
                wkvT = [ph1.tile([128, 1024], bf16, tag=f"wkvT{c}",
                                 name=f"wkvT{c}") for c in range(NCC)]
                wr_sb = [ph1.tile([128, NH], bf16, tag=f"wr{c}", name=f"wr{c}")
                         for c in range(NCC)]
                for c in range(NCC):
                    nc.sync.dma_start(out=wr_sb[c], in_=wrT_d[c])

                # pairwise AllGather reconstructs full weight slices
                nc.sync.dma_start(out=whb_d[:, :, :], in_=wh_d[:, :, :])
                nc.gpsimd.collective_compute(
                    "AllGather", mybir.AluOpType.bypass,
                    replica_groups=[[0, 4], [1, 5], [2, 6], [3, 7]],
                    ins=[whb_d.opt()], outs=[wg_d.opt()])
                wq_blk = list(range(0, 8)) + list(range(20, 28))
                wkv_blk = list(range(8, 12)) + list(range(28, 32))
                wo_blk = list(range(12, 20)) + list(range(32, 40))

                with tc.tile_pool(name="stage", bufs=2) as stage, \
                     tc.tile_pool(name="tps", bufs=4, space="PSUM") as tps:
                    eng = [nc.scalar.copy, nc.vector.tensor_copy]
                    ei = 0
                    for blocks, dsts in (
                            ([xg_d[i] for i in range(16)], xT),
                            ([wg_d[i] for i in wq_blk], wqT),
                            ([wg_d[i] for i in wkv_blk], wkvT),
                            ([wg_d[i] for i in wo_blk], woT)):
                        nb = len(blocks)
                        st_t = stage.tile([128, 16 * CSL], bf16, tag="stg",
                                          name="stg")
                        for sb in range(nb):
                            nc.sync.dma_start(
                                out=st_t[:, sb * CSL:(sb + 1) * CSL],
                                in_=blocks[sb])
                        for cc in range(NCC):
                            if dsts is woT:
                                dst = woT[cc]          # head h = hd-chunk cc
                            else:
                                dst = dsts[cc]
                            for grp in range(nb // 4):
                                pt = tps.tile([128, 512], bf16, tag="tp",
                                              name="tp")
                                for j in range(4):
                                    sb = grp * 4 + j
                                    nc.tensor.transpose(
                                        pt[:, j * 128:(j + 1) * 128],
                                        st_t[:, sb * CSL + cc * 128:
                                             sb * CSL + cc * 128 + 128],
                                        id_b)
                                eng[ei % 2](dst[:, grp * 512:(grp + 1) * 512],
                                            pt)
                                ei += 1

                # ===== phase 1: partial projections for all 16 heads =====
                with tc.tile_pool(name="pps", bufs=4, space="PSUM") as ppool, \
                     tc.tile_pool(name="gps", bufs=2, space="PSUM") as gpool, \
                     tc.tile_pool(name="pev", bufs=6) as epool:
                    ei = 0
                    for st in range(NST):
                        sl = slice(st * STRIP, (st + 1) * STRIP)
                        # q blocks (head h) and kv blocks
                        for ob in range(24):
                            pp = ppool.tile([128, STRIP], f32, tag="pp",
                                            name="pp")
                            for cc in range(NCC):
                                if ob < 16:
                                    stat = wqT[cc][:, ob * 128:(ob + 1) * 128]
                                else:
                                    kb = ob - 16
                                    stat = wkvT[cc][:, kb * 128:(kb + 1) * 128]
                                nc.tensor.matmul(pp, stat, xT[cc][:, sl],
                                                 start=(cc == 0),
                                                 stop=(cc == NCC - 1))
                            ev = epool.tile([128, STRIP], bf16, tag="ev",
                                            name="ev")
                            eng[ei % 2](ev, pp)
                            ei += 1
                            if ob < 16:          # q head ob
                                gg, r0 = ob // GQ, PQ + (ob % GQ) * 128
                            elif ob < 20:        # k head ob-16
                                gg, r0 = ob - 16, PK
                            else:                # v head ob-20
                                gg, r0 = ob - 20, PV
                            nc.sync.dma_start(
                                out=part_d[gg, r0:r0 + 128, sl], in_=ev)
                        # gate logits for all 16 heads
                        gp = gpool.tile([NH, STRIP], f32, tag="gp", name="gp")
                        for cc in range(NCC):
                            nc.tensor.matmul(gp, wr_sb[cc], xT[cc][:, sl],
                                             start=(cc == 0),
                                             stop=(cc == NCC - 1))
                        gev = epool.tile([NH, STRIP], bf16, tag="gev",
                                         name="gev")
                        nc.vector.tensor_copy(gev, gp)
                        for gg in range(GQ):
                            nc.sync.dma_start(
                                out=part_d[gg, PGT:PGT + GQ, sl],
                                in_=gev[gg * GQ:(gg + 1) * GQ, :])

            # ===== ReduceScatter #1: complete q/k/v/gate for own head group ====
            nc.gpsimd.collective_compute(
                "ReduceScatter", mybir.AluOpType.add, replica_groups=GROUPS,
                ins=[part_d.opt()], outs=[rs_d.opt()])

            # ===== phase 1.5: sigmoid gate, RoPE, v transposes, global k/v ====
            with tc.tile_pool(name="rtmp", bufs=3) as rpool, \
                 tc.tile_pool(name="rsst", bufs=1) as rsst, \
                 tc.tile_pool(name="aps", bufs=2, space="PSUM") as apsp, \
                 tc.tile_pool(name="vtps", bufs=2, space="PSUM") as vpp:
                # stage the bf16 RS output, upconvert to f32r working tiles
                qs_bf = rsst.tile([128, GQ * S], bf16, tag="qs_bf",
                                  name="qs_bf")
                kv_bf = rsst.tile([128, 2 * S], bf16, tag="kv_bf",
                                  name="kv_bf")
                g_bfs = rsst.tile([GQ, S], bf16, tag="g_bfs", name="g_bfs")
                glogit = rsst.tile([GQ, S], f32, tag="glogit", name="glogit")
                for h in range(GQ):
                    nc.sync.dma_start(
                        out=qs_bf[:, h * S:(h + 1) * S],
                        in_=rs_d[PQ + h * 128:PQ + (h + 1) * 128, :])
                nc.sync.dma_start(out=kv_bf[:, 0:S], in_=rs_d[PK:PK + 128, :])
                nc.sync.dma_start(out=kv_bf[:, S:2 * S],
                                  in_=rs_d[PV:PV + 128, :])
                nc.sync.dma_start(out=g_bfs, in_=rs_d[PGT:PGT + GQ, :])
                for h in range(GQ):
                    eng[h % 2](qTh_view[:, h, :], qs_bf[:, h * S:(h + 1) * S])
                nc.scalar.copy(kT, kv_bf[:, 0:S])
                nc.vector.tensor_copy(vT, kv_bf[:, S:2 * S])
                nc.scalar.copy(glogit, g_bfs)
                nc.scalar.activation(gateS, glogit, SIGMOID, bias=br_t,
                                     scale=1.0)
                for h in range(GQ):
                    nc.sync.dma_start(out=_grow(h), in_=gateS[h:h + 1, :])
                # RoPE k (in place): k = k*cos + swap(k)*[-sin; sin].
                # The swapped-half copy comes from a partition-shifting
                # SBUF DMA so every DVE operand pair is partition-aligned.
                swp = rpool.tile([128, S], f32, tag="swp", name="swp")
                nc.sync.dma_start(out=swp[0:64], in_=kT[64:128].bitcast(f32))
                nc.sync.dma_start(out=swp[64:128], in_=kT[0:64].bitcast(f32))
                tmp = rpool.tile([128, S], f32, tag="ropetmp", name="ropetmp")
                nc.vector.tensor_mul(tmp, swp, sin2s)
                nc.vector.tensor_mul(kT, kT, cos2)
                nc.gpsimd.tensor_add(kT, kT, tmp)
                # RoPE q + sigmoid gate fold (PE-broadcast gate rows)
                for h in range(GQ):
                    qsl = qTh_view[:, h, :]
                    swp = rpool.tile([128, S], f32, tag="swp", name="swp")
                    nc.sync.dma_start(out=swp[0:64],
                                      in_=qsl[64:128].bitcast(f32))
                    nc.sync.dma_start(out=swp[64:128],
                                      in_=qsl[0:64].bitcast(f32))
                    tmp = rpool.tile([128, S], f32, tag="ropetmp",
                                     name="ropetmp")
                    nc.vector.tensor_mul(tmp, swp, sin2s)
                    nc.vector.tensor_mul(qsl, qsl, cos2)
                    nc.gpsimd.tensor_add(qsl, qsl, tmp)
                    base = (h % 2) * 64
                    for st in range(NST):
                        sl = slice(st * STRIP, (st + 1) * STRIP)
                        a_ps = apsp.tile([128, STRIP], f32, tag="aps",
                                         name="a_ps")
                        nc.tensor.matmul(a_ps, ones1_r[base:base + 1, :],
                                         _grow(h, sl), start=True, stop=True)
                        nc.vector.tensor_mul(qTh_view[:, h, sl],
                                             qTh_view[:, h, sl], a_ps)
                # v transposes: 4 per PSUM bank, 4 wide evacuations
                for grp in range(4):
                    vp = vpp.tile([128, 512], f32, tag="vtp", name="vtp")
                    for j in range(4):
                        c = grp * 4 + j
                        nc.tensor.transpose(vp[:, j * 128:(j + 1) * 128],
                                            vT[:, c * 128:(c + 1) * 128], id_f)
                    dst = v_bf[:, grp * 512:(grp + 1) * 512]
                    if grp % 2 == 0:
                        nc.scalar.copy(dst, vp)
                    else:
                        nc.vector.tensor_copy(dst, vp)
                # dense copies of the strided global k/v slices
                vgs = rpool.tile([128, NG], f32, tag="vgs", name="vgs")
                nc.scalar.copy(vgs, vT[:, 0:S:STRIDE])
                nc.scalar.copy(kg, kT[:, 0:S:STRIDE])
                vgp = vpp.tile([32, 128], f32, tag="vgtp", name="vgtp", bufs=1)
                nc.tensor.transpose(vgp, vgs, id_f)
                nc.scalar.copy(vg_bf, vgp)

            # ============ phase 2: attention + output projection ============
            with tc.tile_pool(name="att", bufs=4) as apool, \
                 tc.tile_pool(name="atts", bufs=2) as spool, \
                 tc.tile_pool(name="outp", bufs=4) as opool, \
                 tc.tile_pool(name="ps_s", bufs=3, space="PSUM") as pss, \
                 tc.tile_pool(name="ps_l", bufs=1, space="PSUM") as psl, \
                 tc.tile_pool(name="ps_av", bufs=2, space="PSUM") as psav, \
                 tc.tile_pool(name="ps_wo", bufs=2, space="PSUM") as pswo:
                for qt in range(NQT):
                    q0 = qt * 128
                    wstart, w = _win(qt)
                    nch = w // 128
                    qrhs = qTh_view[:, :, q0:q0 + 128]        # [128, GQ, 128]
                    l_ps = psl.tile([64, 512], f32, tag="lps", name="lps")
                    # ---- local chunks: S^T, mask, exp, l, AV ----
                    av_l = psav.tile([128, 512], f32, tag="av", name="av_l")
                    for c in range(nch):
                        kc = wstart // 128 + c
                        ksl = slice(kc * 128, (kc + 1) * 128)
                        sp = pss.tile([128, 512], f32, tag="sps", name="sps")
                        last = (qt == 0) or (c == nch - 1) or (qt >= 2 and c == 0)
                        nc.tensor.matmul(sp, kT[:, ksl], qrhs,
                                         start=True, stop=not last)
                        if qt >= 2 and c == 0:
                            nc.tensor.matmul(sp, id_r, m_lo, start=False,
                                             stop=True)
                        elif c == nch - 1:
                            nc.tensor.matmul(sp, id_r, m_hi, start=False,
                                             stop=True)
                        pT = apool.tile([128, 512], bf16, tag="pT", name="pT")
                        nc.scalar.activation(pT, sp, EXP, scale=SCALE)
                        nc.tensor.matmul(l_ps[0:1, :], w07_bf, pT,
                                         start=(c == 0), stop=(c == nch - 1))
                        nc.tensor.matmul(av_l, v_bf[:, ksl], pT,
                                         start=(c == 0), stop=(c == nch - 1))
                    # ---- global: S^T_g, exp, l_g, AV_g ----
                    spg = pss.tile([32, 512], f32, tag="sps", name="spg")
                    nc.tensor.matmul(spg, kg, qrhs, start=True, stop=True)
                    pTg = apool.tile([32, 512], bf16, tag="pTg", name="pTg")
                    nc.scalar.activation(pTg, spg, EXP, scale=SCALE)
                    nc.tensor.matmul(l_ps[32:33, :], w03_bf[0:32, :], pTg,
                                     start=True, stop=True)
                    av_g = psav.tile([128, 512], f32, tag="av", name="av_g")
                    nc.tensor.matmul(av_g, vg_bf, pTg, start=True, stop=True)
                    # ---- normalization + 0.7/0.3 mix ----
                    r_l = spool.tile([1, 512], f32r, tag="r_l", name="r_l")
                    r_g = spool.tile([1, 512], f32r, tag="r_g", name="r_g")
                    with nc.allow_low_precision("f32r == f32 bits"):
                        nc.vector.reciprocal(r_l, l_ps[0:1, :])
                        nc.vector.reciprocal(r_g, l_ps[32:33, :])
                    rbp_l = pss.tile([128, 512], f32, tag="sps", name="rbp_l")
                    nc.tensor.matmul(rbp_l, ones1_r[0:1, :], r_l,
                                     start=True, stop=True)
                    rbp_g = pss.tile([128, 512], f32, tag="sps", name="rbp_g")
                    nc.tensor.matmul(rbp_g, ones1_r[0:1, :], r_g,
                                     start=True, stop=True)
                    rb_l = spool.tile([128, 512], f32, tag="rb_l", name="rb_l")
                    rb_g = spool.tile([128, 512], f32, tag="rb_g", name="rb_g")
                    nc.scalar.copy(rb_l, rbp_l)
                    nc.vector.tensor_copy(rb_g, rbp_g)
                    t_l = spool.tile([128, 512], f32, tag="t_l", name="t_l")
                    t_g = spool.tile([128, 512], f32, tag="t_g", name="t_g")
                    nc.vector.tensor_mul(t_l, av_l, rb_l)
                    nc.vector.tensor_mul(t_g, av_g, rb_g)
                    at_all = spool.tile([128, 512], bf16, tag="at", name="at",
                                        bufs=3)
                    nc.gpsimd.tensor_add(at_all, t_l, t_g)
                    # ---- output projection partial for this q tile ----
                    for os_ in range(4):
                        osl = slice(os_ * 512, (os_ + 1) * 512)
                        wo_ps = pswo.tile([128, 512], f32, tag="wops",
                                          name="wops")
                        for h in range(GQ):
                            nc.tensor.matmul(wo_ps,
                                             at_all[:, h * 128:(h + 1) * 128],
                                             woT[h][:, osl],
                                             start=(h == 0), stop=(h == GQ - 1))
                        ot = opool.tile([128, 512], bf16, tag="ot", name="ot")
                        if os_ % 2 == 0:
                            nc.scalar.copy(ot, wo_ps)
                        else:
                            nc.vector.tensor_copy(ot, wo_ps)
                        nc.sync.dma_start(
                            out=po_d[qt // 4, (qt % 4) * 128:(qt % 4) * 128 + 128,
                                     osl],
                            in_=ot)

            # ===== ReduceScatter #2: sum head groups, scatter over seq =====
            nc.gpsimd.collective_compute(
                "ReduceScatter", mybir.AluOpType.add, replica_groups=GROUPS,
                ins=[po_d.opt()], outs=[ro_d.opt()])
            nc.sync.dma_start(out=out_d[:, :], in_=ro_d[:, :])

    nc.finalize()
    return nc


_NC_CACHE = {}


def _get_nc():
    if "nc" not in _NC_CACHE:
        _NC_CACHE["nc"] = _build_nc()
    return _NC_CACHE["nc"]


def _prep_core(args):
    x, Wq, Wkv, Wo, Wr, br, c = args
    bf = ml_dtypes.bfloat16
    b, g = divmod(c, 4)
    cols = slice(g * CSL, (g + 1) * CSL)
    # this core's half of the weight slices (other half comes from the
    # paired core via an on-device AllGather)
    wh = np.empty((20, 128, CSL), dtype=bf)
    whf = wh.reshape(20 * 128, CSL)
    whf[0:1024] = Wq[b * 1024:(b + 1) * 1024, cols]
    whf[1024:1536] = Wkv[b * 512:(b + 1) * 512, cols]
    whf[1536:2560] = Wo[b * 1024:(b + 1) * 1024, cols]
    xg = np.empty((16, 128, CSL), dtype=bf)
    xg.reshape(S, CSL)[:] = x[b][:, cols]
    return {
        "xg": xg,
        "wh": wh,
        "wrT": np.ascontiguousarray(Wr[:, cols].T.astype(bf)
                                    ).reshape(NCC, 128, NH),
        "br": np.ascontiguousarray(br[g * GQ:(g + 1) * GQ].reshape(GQ, 1)),
    }


def kernel(x, Wq, Wkv, Wo, Wr, br):
    from concurrent.futures import ThreadPoolExecutor
    x = np.asarray(x)
    Wq = np.asarray(Wq)
    Wkv = np.asarray(Wkv)
    Wo = np.asarray(Wo)
    Wr = np.asarray(Wr, dtype=np.float32)
    br = np.asarray(br, dtype=np.float32)

    nc = _get_nc()
    with ThreadPoolExecutor(max_workers=8) as tp:
        in_maps = list(tp.map(_prep_core,
                              [(x, Wq, Wkv, Wo, Wr, br, c)
                               for c in range(8)]))
    res = run_bass_kernel_spmd(nc, in_maps, list(range(8)))
    out = np.empty((B, S, DIM), dtype=np.float32)

    def _place(c):
        b, g = divmod(c, 4)
        out[b, g * STRIP:(g + 1) * STRIP, :] = \
            res.results[c]["out"].astype(np.float32)
    with ThreadPoolExecutor(max_workers=8) as tp:
        list(tp.map(_place, range(8)))
    return out
